# revision 1
# baseline (speedup 1.0000x reference)
"""CrossGraphAttentionModel on 8 Trainium2 NeuronCores (Bass/Tile, SPMD).

Sharding: nodes/edges of both graphs are sharded 8 ways by (dst-sorted) node
range; 64-dim weights replicated. Per GINE layer each core dma_gathers x[src]
for its edge shard from an AllGathered copy of x in HBM, forms messages on
DVE/ACT, and scatter-adds them with one-hot matmuls on the PE (PSUM
accumulation), then runs the node MLP on its node shard and AllGathers the new
x. Cross-graph attention shards the query axis: scores are computed twice on
PE - once [q,k] for an exact row max, once [k,q] with the max folded into the
contraction via an appended ones row - so softmax needs only a single ACT exp
pass, and the exp tiles feed the wV matmul directly as lhsT with a ones column
in V producing the softmax denominator for free. Graph pooling is a one-hot
matmul with 1/count weights, AllReduced, followed by the tiny output MLP.

All floating point math runs on device in fp32; the host only sorts/pads
integer index structures and transposes/replicates input layouts.
"""

import numpy as np

R = 8
HID = 64
B = 32
HEADS = 4
HD = 16
N_MOL, N_PROT = 2048, 4096
E_MOL, E_PROT = 32768, 131072
NC_MOL, NC_PROT = N_MOL // R, N_PROT // R          # 256, 512
NBLK_MOL, NBLK_PROT = NC_MOL // 128, NC_PROT // 128  # 2, 4

_CACHE = {}
last_results = None


# ----------------------------------------------------------------- host prep

def _prep_edges(edge_index, eattr, nblk):
    """Sort edges by dst, partition into R cores x nblk 128-node windows,
    pad every window to T_blk tiles of 128 edges. Returns device layouts."""
    src, dst = np.asarray(edge_index[0]), np.asarray(edge_index[1])
    eattr = np.asarray(eattr, np.float32)
    order = np.argsort(dst, kind="stable")
    src_s, dst_s, ea_s = src[order], dst[order], eattr[order]
    nblocks = R * nblk
    blk = dst_s // 128
    counts = np.bincount(blk, minlength=nblocks)
    T_blk = int(np.ceil(counts.max() / 128))
    T_total = nblk * T_blk
    E_core = T_total * 128
    D = eattr.shape[1]

    gidx = np.zeros((R, E_core), np.int64)
    dstoff = np.full((R, E_core), -1.0, np.float32)
    ea_pad = np.zeros((R, E_core, D), np.float32)
    starts = np.concatenate([[0], np.cumsum(counts)])
    for c in range(R):
        for b in range(nblk):
            g = c * nblk + b
            cnt = counts[g]
            lo = starts[g]
            off = b * T_blk * 128
            gidx[c, off:off + cnt] = src_s[lo:lo + cnt]
            dstoff[c, off:off + cnt] = (dst_s[lo:lo + cnt] - g * 128)
            ea_pad[c, off:off + cnt] = ea_s[lo:lo + cnt]

    # gather indices wrapped [128, E_core/16] (i -> p=i%16, col=i//16), x8 replicated
    cols = E_core // 16
    w = gidx.reshape(R, cols, 16).transpose(0, 2, 1).astype(np.int16)
    gidx_sb = np.tile(w, (1, 8, 1)).copy()
    # dstoff [128, T_total]
    dstoff_sb = np.ascontiguousarray(
        dstoff.reshape(R, T_total, 128).transpose(0, 2, 1))
    # eattr^T with ones row: [11, E_core]
    eaT_packed = np.ascontiguousarray(np.concatenate(
        [ea_pad.transpose(0, 2, 1),
         np.ones((R, 1, E_core), np.float32)], axis=1))
    return dict(T_blk=T_blk, T_total=T_total, E_core=E_core, D=D,
                gidx_sb=gidx_sb, dstoff_sb=dstoff_sb, eaT_packed=eaT_packed)


def _prep_host(inp):
    """All integer/layout preprocessing. Returns (meta, per_core_inputs)."""
    mol = _prep_edges(inp["mol_edge_index"], inp["mol_eattr"], NBLK_MOL)
    prot = _prep_edges(inp["prot_edge_index"], inp["prot_eattr"], NBLK_PROT)

    # pool matrices with 1/count entries
    def pmat(batch, ncore):
        batch = np.asarray(batch)
        cnt = np.bincount(batch, minlength=B).astype(np.float32)
        inv = 1.0 / np.maximum(cnt, 1.0)
        m = np.zeros((R, ncore, B), np.float32)
        for c in range(R):
            sl = batch[c * ncore:(c + 1) * ncore]
            m[c, np.arange(ncore), sl] = inv[sl]
        return m

    mol_pmat = pmat(inp["mol_batch"], NC_MOL)
    prot_pmat = pmat(inp["prot_batch"], NC_PROT)

    # node features transposed per core with ones row
    def xt(x, ncore):
        x = np.asarray(x, np.float32)
        d = x.shape[1]
        out = np.zeros((R, d + 1, ncore), np.float32)
        for c in range(R):
            out[c, :d] = x[c * ncore:(c + 1) * ncore].T
            out[c, d] = 1.0
        return out

    mol_xT = xt(inp["mol_x"], NC_MOL)        # [R, 12, 256]
    prot_xT = xt(inp["prot_x"], NC_PROT)     # [R, 16, 512]

    iota = np.tile(np.arange(128, dtype=np.float32), (128, 1))
    ident = np.eye(128, dtype=np.float32)

    # attn K-bias as per-head columns [16, 4]
    def bcols(b):  # [64] -> [16, 4]
        return np.ascontiguousarray(np.asarray(b, np.float32).reshape(4, 16).T)

    percore = []
    for c in range(R):
        m = {
            "mol_xT": mol_xT[c], "prot_xT": prot_xT[c],
            "mol_eaT": mol["eaT_packed"][c], "prot_eaT": prot["eaT_packed"][c],
            "mol_gidx": mol["gidx_sb"][c], "prot_gidx": prot["gidx_sb"][c],
            "mol_dstoff": mol["dstoff_sb"][c], "prot_dstoff": prot["dstoff_sb"][c],
            "mol_pmat": mol_pmat[c], "prot_pmat": prot_pmat[c],
            "iota": iota, "ident": ident,
            "bk_mp_cols": bcols(np.asarray(inp["attn_mp_b"])[1]),
            "bk_pm_cols": bcols(np.asarray(inp["attn_pm_b"])[1]),
        }
        for k in ("node_lin_mol_W", "node_lin_mol_b", "node_lin_prot_W",
                  "node_lin_prot_b", "edge_lin_mol_W", "edge_lin_mol_b",
                  "edge_lin_prot_W", "edge_lin_prot_b",
                  "mol_conv_W1", "mol_conv_b1", "mol_conv_W2", "mol_conv_b2",
                  "prot_conv_W1", "prot_conv_b1", "prot_conv_W2", "prot_conv_b2",
                  "attn_mp_W", "attn_mp_b", "attn_pm_W", "attn_pm_b",
                  "fc1_W", "fc1_b", "fc2_W", "fc2_b"):
            m[k] = np.asarray(inp[k], np.float32)
        percore.append(m)

    meta = dict(mol_T_blk=mol["T_blk"], mol_T_total=mol["T_total"],
                mol_E_core=mol["E_core"],
                prot_T_blk=prot["T_blk"], prot_T_total=prot["T_total"],
                prot_E_core=prot["E_core"])
    return meta, percore


# ------------------------------------------------------------- device build

def _build(meta):
    import concourse.bacc as bacc
    import concourse.mybir as mybir
    import concourse.tile as tile

    F32 = mybir.dt.float32
    I16 = mybir.dt.int16
    AF = mybir.ActivationFunctionType
    ALU = mybir.AluOpType

    nc = bacc.Bacc("TRN2", target_bir_lowering=False, debug=False,
                   num_devices=R)

    # ---- I/O declarations
    dram = {}

    def din(name, shape, dtype=F32):
        dram[name] = nc.dram_tensor(name, list(shape), dtype,
                                    kind="ExternalInput")
        return dram[name]

    mT, mE = meta["mol_T_total"], meta["mol_E_core"]
    pT, pE = meta["prot_T_total"], meta["prot_E_core"]

    din("mol_xT", [12, NC_MOL]); din("prot_xT", [16, NC_PROT])
    din("mol_eaT", [11, mE]); din("prot_eaT", [11, pE])
    din("mol_gidx", [128, mE // 16], I16); din("prot_gidx", [128, pE // 16], I16)
    din("mol_dstoff", [128, mT]); din("prot_dstoff", [128, pT])
    din("mol_pmat", [NC_MOL, B]); din("prot_pmat", [NC_PROT, B])
    din("iota", [128, 128]); din("ident", [128, 128])
    din("bk_mp_cols", [16, 4]); din("bk_pm_cols", [16, 4])
    din("node_lin_mol_W", [11, 64]); din("node_lin_mol_b", [64])
    din("node_lin_prot_W", [15, 64]); din("node_lin_prot_b", [64])
    din("edge_lin_mol_W", [10, 64]); din("edge_lin_mol_b", [64])
    din("edge_lin_prot_W", [10, 64]); din("edge_lin_prot_b", [64])
    for s in ("mol", "prot"):
        din(f"{s}_conv_W1", [3, 64, 64]); din(f"{s}_conv_b1", [3, 64])
        din(f"{s}_conv_W2", [3, 64, 64]); din(f"{s}_conv_b2", [3, 64])
    din("attn_mp_W", [3, 64, 64]); din("attn_mp_b", [3, 64])
    din("attn_pm_W", [3, 64, 64]); din("attn_pm_b", [3, 64])
    din("fc1_W", [128, 64]); din("fc1_b", [64])
    din("fc2_W", [64, 1]); din("fc2_b", [1])

    out_d = nc.dram_tensor("out", [1, B], F32, kind="ExternalOutput")

    sides = {
        "mol": dict(N=N_MOL, NC=NC_MOL, nblk=NBLK_MOL, T_blk=meta["mol_T_blk"],
                    T_total=mT, E_core=mE, D=10),
        "prot": dict(N=N_PROT, NC=NC_PROT, nblk=NBLK_PROT,
                     T_blk=meta["prot_T_blk"], T_total=pT, E_core=pE, D=10),
    }

    with tile.TileContext(nc) as tc:
        # ---------------- persistent SBUF constants
        const = tc.alloc_tile_pool(name="const", bufs=1)

        def load_const(name, shape, dtype=F32, src=None):
            t = const.tile(list(shape), dtype, name=f"c_{name}")
            nc.sync.dma_start(t[:], (dram[name] if src is None else src)[:])
            return t

        iota_sb = load_const("iota", [128, 128])
        ident_sb = load_const("ident", [128, 128])

        def wcat(name_w, name_b, din_, dout, wslice=None, bslice=None):
            t = const.tile([din_ + 1, dout], F32, name=f"w_{name_w}_{wslice}")
            wsrc = dram[name_w] if wslice is None else dram[name_w][wslice]
            bsrc = dram[name_b] if bslice is None else dram[name_b][bslice]
            nc.sync.dma_start(t[0:din_, :], wsrc[:, :] if wslice is None else wsrc)
            nc.sync.dma_start(t[din_:din_ + 1, :], bsrc[None, :])
            return t

        Wn = {"mol": wcat("node_lin_mol_W", "node_lin_mol_b", 11, 64),
              "prot": wcat("node_lin_prot_W", "node_lin_prot_b", 15, 64)}
        We = {"mol": wcat("edge_lin_mol_W", "edge_lin_mol_b", 10, 64),
              "prot": wcat("edge_lin_prot_W", "edge_lin_prot_b", 10, 64)}
        W1 = {s: [wcat(f"{s}_conv_W1", f"{s}_conv_b1", 64, 64, l, l)
                  for l in range(3)] for s in ("mol", "prot")}
        W2 = {s: [wcat(f"{s}_conv_W2", f"{s}_conv_b2", 64, 64, l, l)
                  for l in range(3)] for s in ("mol", "prot")}

        sb_idx, sb_dstoff = {}, {}
        for s in sides:
            sd = sides[s]
            sb_idx[s] = load_const(f"{s}_gidx", [128, sd["E_core"] // 16], I16)
            sb_dstoff[s] = load_const(f"{s}_dstoff", [128, sd["T_total"]])
        sb_xTin = {"mol": load_const("mol_xT", [12, NC_MOL]),
                   "prot": load_const("prot_xT", [16, NC_PROT])}
        sb_pmat = {}
        for s in sides:
            sd = sides[s]
            t = const.tile([128, sd["nblk"], B], F32, name=f"pmat_{s}")
            nc.sync.dma_start(
                t[:], dram[f"{s}_pmat"].rearrange("(t p) g -> p t g", p=128))
            sb_pmat[s] = t

        # ---------------- DRAM internals
        dpool = tc.alloc_tile_pool(name="dram", bufs=1, space="DRAM")
        x_sh_d = {s: [dpool.tile([sides[s]["NC"], 64], F32,
                                 name=f"xsh_{s}_{l}") for l in range(4)]
                  for s in sides}
        x_full_d = {s: [dpool.tile([sides[s]["N"], 64], F32,
                                   addr_space="Shared", name=f"xfull_{s}_{l}")
                        for l in range(4)] for s in sides}

        # ---------------- long-lived x pools, then GINE-scoped pools
        xT_pool = tc.alloc_tile_pool(name="xT", bufs=2)
        xnf_pool = tc.alloc_tile_pool(name="xnf", bufs=2)
        gmem = tc.alloc_tile_pool(name="gmem", bufs=1)
        empp = tc.alloc_tile_pool(name="empp", bufs=2, space="PSUM")
        aggps = tc.alloc_tile_pool(name="aggps", bufs=2, space="PSUM")
        mlpps = tc.alloc_tile_pool(name="mlpps", bufs=2, space="PSUM")
        trps = tc.alloc_tile_pool(name="trps", bufs=2, space="PSUM")

        # edge features em = [eattr;1] @ [We;be], edge-major [128, T, 64],
        # eattr^T streamed from DRAM per block
        ea_stream = tc.alloc_tile_pool(name="ea_stream", bufs=2)
        em_sb = {}
        for s in sides:
            sd = sides[s]
            T_total, T_blk, nblk, D = (sd["T_total"], sd["T_blk"], sd["nblk"],
                                       sd["D"])
            em = gmem.tile([128, T_total, 64], F32, name=f"em_{s}")
            for b in range(nblk):
                ch = ea_stream.tile([11, T_blk * 128], F32, name="ea_chunk")
                nc.sync.dma_start(
                    ch[:],
                    dram[f"{s}_eaT"][:, b * T_blk * 128:(b + 1) * T_blk * 128])
                for t0 in range(0, T_blk, 8):
                    ng = min(8, T_blk - t0)
                    ps = empp.tile([128, 8, 64], F32, name="em_ps")
                    for j in range(ng):
                        nc.tensor.matmul(
                            ps[:, j, :],
                            ch[0:D + 1, (t0 + j) * 128:(t0 + j + 1) * 128],
                            We[s][:], start=True, stop=True)
                    nc.vector.tensor_copy(
                        em[:, b * T_blk + t0:b * T_blk + t0 + ng, :],
                        ps[:, 0:ng, :])
            em_sb[s] = em
        ea_stream.release()
        xg_pool = tc.alloc_tile_pool(name="xg", bufs=2)
        oh_pool = tc.alloc_tile_pool(name="oh", bufs=2)

        # initial node features x0
        xT_cur = {}
        xnf_cur = {}
        for s in sides:
            sd = sides[s]
            NCs, nblk = sd["NC"], sd["nblk"]
            ps = mlpps.tile([64, 512], F32, name="mlp_ps")
            nc.tensor.matmul(ps[:, 0:NCs], Wn[s][:], sb_xTin[s][:],
                             start=True, stop=True)
            xT = xT_pool.tile([65, NCs], F32, name=f"xT_{s}")
            nc.vector.tensor_copy(xT[0:64, :], ps[:, 0:NCs])
            nc.vector.memset(xT[64:65, :], 1.0)
            xnf = xnf_pool.tile([128, nblk, 64], F32, name=f"xnf_{s}")
            for b in range(nblk):
                tp = trps.tile([128, 64], F32, name="tr_ps")
                nc.tensor.transpose(tp[:], xT[0:64, b * 128:(b + 1) * 128],
                                    ident_sb[0:64, 0:64])
                nc.vector.tensor_copy(xnf[:, b, :], tp[:])
            nc.sync.dma_start(
                x_sh_d[s][0][:].rearrange("(t p) f -> p t f", p=128), xnf[:])
            nc.gpsimd.collective_compute(
                "AllGather", ALU.bypass, replica_groups=[list(range(R))],
                ins=[x_sh_d[s][0][:].opt()], outs=[x_full_d[s][0][:].opt()])
            xT_cur[s] = xT
            xnf_cur[s] = xnf

        # GINE layers
        for l in range(3):
            for s in ("prot", "mol"):
                sd = sides[s]
                NCs, nblk, T_blk = sd["NC"], sd["nblk"], sd["T_blk"]
                xT_prev = xT_cur[s]
                hT = gmem.tile([65, NCs], F32, name=f"hT_{s}_{l}", bufs=2,
                               tag=f"hT_{s}")
                for b in range(nblk):
                    nE = T_blk * 128
                    xg = xg_pool.tile([128, T_blk, 64], F32, name="xg")
                    nc.gpsimd.dma_gather(
                        xg[:], x_full_d[s][l][:],
                        sb_idx[s][:, b * T_blk * 8:(b + 1) * T_blk * 8],
                        nE, nE, 64, single_packet=False)
                    msg = xg_pool.tile([128, T_blk, 64], F32, name="msg")
                    nc.vector.tensor_add(
                        msg[:], xg[:], em_sb[s][:, b * T_blk:(b + 1) * T_blk, :])
                    nc.scalar.activation(msg[:], msg[:], AF.Relu)
                    oh = oh_pool.tile([128, T_blk, 128], F32, name="oh")
                    nc.vector.tensor_tensor(
                        oh[:],
                        iota_sb[:, :].unsqueeze(1).broadcast_to([128, T_blk, 128]),
                        sb_dstoff[s][:, b * T_blk:(b + 1) * T_blk]
                            .unsqueeze(2).broadcast_to([128, T_blk, 128]),
                        ALU.is_equal)
                    agg = aggps.tile([64, 128], F32, name="agg_ps")
                    for t in range(T_blk):
                        nc.tensor.matmul(agg[:], msg[:, t, :], oh[:, t, :],
                                         start=(t == 0), stop=(t == T_blk - 1))
                    nc.vector.tensor_add(hT[0:64, b * 128:(b + 1) * 128],
                                         xT_prev[0:64, b * 128:(b + 1) * 128],
                                         agg[:])
                nc.vector.memset(hT[64:65, :], 1.0)
                ps1 = mlpps.tile([64, 512], F32, name="mlp_ps")
                nc.tensor.matmul(ps1[:, 0:NCs], W1[s][l][:], hT[:],
                                 start=True, stop=True)
                r1 = gmem.tile([65, NCs], F32, name=f"r1_{s}_{l}", bufs=2,
                               tag=f"r1_{s}")
                nc.scalar.activation(r1[0:64, :], ps1[:, 0:NCs], AF.Relu)
                nc.vector.memset(r1[64:65, :], 1.0)
                ps2 = mlpps.tile([64, 512], F32, name="mlp_ps")
                nc.tensor.matmul(ps2[:, 0:NCs], W2[s][l][:], r1[:],
                                 start=True, stop=True)
                xT = xT_pool.tile([65, NCs], F32, name=f"xT_{s}")
                nc.scalar.activation(xT[0:64, :], ps2[:, 0:NCs], AF.Relu)
                nc.vector.memset(xT[64:65, :], 1.0)
                xnf = xnf_pool.tile([128, nblk, 64], F32, name=f"xnf_{s}")
                for b in range(nblk):
                    tp = trps.tile([128, 64], F32, name="tr_ps")
                    nc.tensor.transpose(tp[:], xT[0:64, b * 128:(b + 1) * 128],
                                        ident_sb[0:64, 0:64])
                    nc.vector.tensor_copy(xnf[:, b, :], tp[:])
                nc.sync.dma_start(
                    x_sh_d[s][l + 1][:].rearrange("(t p) f -> p t f", p=128),
                    xnf[:])
                nc.gpsimd.collective_compute(
                    "AllGather", ALU.bypass, replica_groups=[list(range(R))],
                    ins=[x_sh_d[s][l + 1][:].opt()],
                    outs=[x_full_d[s][l + 1][:].opt()])
                xT_cur[s] = xT
                xnf_cur[s] = xnf

        # close GINE-scoped pools (LIFO per space)
        oh_pool.release()
        xg_pool.release()
        for p in (trps, mlpps, aggps, empp):
            p.release()
        gmem.release()

        # ---------------- attention phase
        a_sb = tc.alloc_tile_pool(name="attn_sb", bufs=1)
        smallps = tc.alloc_tile_pool(name="smallps", bufs=2, space="PSUM")
        s12ps = tc.alloc_tile_pool(name="s12ps", bufs=2, space="PSUM")
        ops = tc.alloc_tile_pool(name="ops", bufs=4, space="PSUM")
        exp_pool = tc.alloc_tile_pool(name="expt", bufs=10)
        WAVE = 8

        # full x (both sides), transposed with ones row
        xT_full = {}
        for s in sides:
            sd = sides[s]
            Ns = sd["N"]
            nt = Ns // 128
            xT_f = a_sb.tile([65, Ns], F32, name=f"xTfull_{s}")
            for t in range(nt):
                xf_nf = a_sb.tile([128, 64], F32, name="xf_nf", bufs=3,
                                  tag="xf_nf")
                nc.sync.dma_start(
                    xf_nf[:], x_full_d[s][3][t * 128:(t + 1) * 128, :])
                tp = smallps.tile([128, 512], F32, name="small_ps")
                nc.tensor.transpose(tp[0:64, 0:128], xf_nf[:], ident_sb[:])
                nc.vector.tensor_copy(xT_f[0:64, t * 128:(t + 1) * 128],
                                      tp[0:64, 0:128])
            nc.vector.memset(xT_f[64:65, :], 1.0)
            xT_full[s] = xT_f

        H_sb = {}
        for dirn, (qs, ks) in (("mp", ("mol", "prot")), ("pm", ("prot", "mol"))):
            qd, kd = sides[qs], sides[ks]
            NCq, Nk = qd["NC"], kd["N"]
            n_qt = NCq // 128
            n_k512 = Nk // 512
            n_k128 = Nk // 128
            Wd = dram[f"attn_{dirn}_W"]
            bd = dram[f"attn_{dirn}_b"]

            Wq = a_sb.tile([65, 64], F32, name=f"Wq_{dirn}")
            nc.sync.dma_start(Wq[0:64, :], Wd[0])
            nc.sync.dma_start(Wq[64:65, :], bd[0][None, :])
            Wv = a_sb.tile([65, 64], F32, name=f"Wv_{dirn}")
            nc.sync.dma_start(Wv[0:64, :], Wd[2])
            nc.sync.dma_start(Wv[64:65, :], bd[2][None, :])
            Wk_raw = a_sb.tile([64, 64], F32, name=f"Wkraw_{dirn}")
            nc.sync.dma_start(Wk_raw[:], Wd[1])
            bk_cols = a_sb.tile([16, 4], F32, name=f"bkcols_{dirn}")
            nc.sync.dma_start(bk_cols[:], dram[f"bk_{dirn}_cols"][:])

            # rhs0_h = [R_h ; c_h]: folded K-side coefficients per head.
            # s^T chunk = rhs0_h^T @ xT_full  gives [q, k] scores (pass 1);
            # with row 64 -= m_h it gives s~^T in [k, q] (pass 2).
            QT, rhs0 = [], []
            cT = a_sb.tile([1, HEADS, NCq], F32, name=f"cT_{dirn}")
            for h in range(HEADS):
                tp = smallps.tile([128, 512], F32, name="small_ps")
                nc.tensor.transpose(tp[0:16, 0:64],
                                    Wk_raw[:, 16 * h:16 * h + 16],
                                    ident_sb[0:64, 0:64])
                wkt = a_sb.tile([16, 64], F32, name="wkt", bufs=2, tag="wkt")
                nc.vector.tensor_copy(wkt[:], tp[0:16, 0:64])

                ps = smallps.tile([128, 512], F32, name="small_ps")
                nc.tensor.matmul(ps[0:16, 0:NCq],
                                 Wq[:, 16 * h:16 * h + 16], xT_cur[qs][:],
                                 start=True, stop=True)
                qt_ = a_sb.tile([16, NCq], F32, name=f"QT_{dirn}_{h}")
                nc.scalar.activation(qt_[:], ps[0:16, 0:NCq], AF.Copy,
                                     scale=0.25)
                QT.append(qt_)

                psR = smallps.tile([128, 512], F32, name="small_ps")
                nc.tensor.matmul(psR[0:64, 0:NCq], wkt[:], qt_[:],
                                 start=True, stop=True)
                psC = smallps.tile([128, 512], F32, name="small_ps")
                nc.tensor.matmul(psC[0:1, 0:NCq], bk_cols[:, h:h + 1],
                                 qt_[:], start=True, stop=True)
                r0 = a_sb.tile([65, NCq], F32, name=f"rhs0_{dirn}_{h}")
                nc.vector.tensor_copy(r0[0:64, :], psR[0:64, 0:NCq])
                nc.vector.tensor_copy(r0[64:65, :], psC[0:1, 0:NCq])
                nc.vector.tensor_copy(cT[0:1, h, :], psC[0:1, 0:NCq])
                rhs0.append(r0)

            # V' [128, n_k128, 4, 17] with ones col
            Vp = a_sb.tile([128, n_k128, HEADS, 17], F32, name=f"Vp_{dirn}")
            nc.vector.memset(Vp[:, :, :, 16:17], 1.0)
            for kt in range(n_k128):
                ps = smallps.tile([128, 512], F32, name="small_ps")
                nc.tensor.matmul(ps[0:128, 0:64],
                                 xT_full[ks][:, kt * 128:(kt + 1) * 128],
                                 Wv[:], start=True, stop=True)
                nc.vector.tensor_copy(
                    Vp[:, kt, :, 0:16],
                    ps[0:128, 0:64].rearrange("p (h d) -> p h d", h=HEADS))

            # pass 1: exact row max m_h [1, NCq] per head ([q, k] layout)
            mT = a_sb.tile([1, HEADS, NCq], F32, name=f"mT_{dirn}")
            for h in range(HEADS):
                for qt in range(n_qt):
                    mx = a_sb.tile([128, n_k512], F32, name="mx", bufs=2,
                                   tag="mx")
                    for cch in range(n_k512):
                        s1 = s12ps.tile([128, 512], F32, name="s12_ps")
                        nc.tensor.matmul(
                            s1[:],
                            rhs0[h][:, qt * 128:(qt + 1) * 128],
                            xT_full[ks][:, cch * 512:(cch + 1) * 512],
                            start=True, stop=True)
                        nc.vector.reduce_max(mx[:, cch:cch + 1], s1[:],
                                             axis=mybir.AxisListType.X)
                    mqt = a_sb.tile([128, 1], F32, name="mqt", bufs=2, tag="mqt")
                    nc.vector.reduce_max(mqt[:], mx[:], axis=mybir.AxisListType.X)
                    tp = smallps.tile([128, 512], F32, name="small_ps")
                    nc.tensor.transpose(tp[0:1, 0:128], mqt[:], ident_sb[:])
                    nc.vector.tensor_copy(
                        mT[0:1, h, qt * 128:(qt + 1) * 128], tp[0:1, 0:128])

            # pass 2 + wV, processed in waves of WAVE k-chunks
            H = a_sb.tile([128, n_qt, 64], F32, name=f"H_{dirn}")
            for h in range(HEADS):
                rhs = a_sb.tile([65, NCq], F32, name="rhs", bufs=2, tag="rhs")
                nc.vector.tensor_copy(rhs[0:64, :], rhs0[h][0:64, :])
                cm = a_sb.tile([1, NCq], F32, name="cm", bufs=2, tag="cm")
                nc.vector.tensor_sub(cm[:], cT[0:1, h, :], mT[0:1, h, :])
                nc.vector.tensor_copy(rhs[64:65, :], cm[:])
                o_tiles = [ops.tile([128, 17], F32, name="o_ps")
                           for _ in range(n_qt)]
                for w0 in range(0, n_k128, WAVE):
                    nw = min(WAVE, n_k128 - w0)
                    exs = []
                    for j in range(nw):
                        kc = w0 + j
                        s2 = s12ps.tile([128, 512], F32, name="s12_ps")
                        nc.tensor.matmul(
                            s2[:, 0:NCq],
                            xT_full[ks][:, kc * 128:(kc + 1) * 128],
                            rhs[:], start=True, stop=True)
                        ex = exp_pool.tile([128, NCq], F32, name="ex",
                                           tag=f"ex_{dirn}")
                        nc.scalar.activation(ex[:], s2[:, 0:NCq], AF.Exp)
                        exs.append(ex)
                    for qt in range(n_qt):
                        for j in range(nw):
                            kc = w0 + j
                            nc.tensor.matmul(
                                o_tiles[qt][:],
                                exs[j][:, qt * 128:(qt + 1) * 128],
                                Vp[:, kc, h, :],
                                start=(kc == 0), stop=(kc == n_k128 - 1))
                for qt in range(n_qt):
                    inv1 = a_sb.tile([128, 1], F32, name="inv1", bufs=2,
                                     tag="inv1")
                    nc.vector.reciprocal(inv1[:], o_tiles[qt][:, 16:17])
                    nc.vector.tensor_scalar_mul(
                        H[:, qt, 16 * h:16 * (h + 1)], o_tiles[qt][:, 0:16],
                        inv1[:])

            # residual: H += x (node-major shard)
            nc.vector.tensor_add(H[:], H[:], xnf_cur[qs][:])
            H_sb[dirn] = H

        # ---------------- pooling + output MLP
        zt_part_d = dpool.tile([128, B], F32, name="zt_part")
        zt_full_d = dpool.tile([128, B], F32, addr_space="Shared",
                               name="zt_full")
        for dirn, qs in (("mp", "mol"), ("pm", "prot")):
            n_qt = sides[qs]["NC"] // 128
            psz = smallps.tile([128, 512], F32, name="small_ps")
            for qt in range(n_qt):
                nc.tensor.matmul(psz[0:64, 0:B], H_sb[dirn][:, qt, :],
                                 sb_pmat[qs][:, qt, :],
                                 start=(qt == 0), stop=(qt == n_qt - 1))
            zpart = a_sb.tile([64, B], F32, name=f"zpart_{dirn}")
            nc.vector.tensor_copy(zpart[:], psz[0:64, 0:B])
            row0 = 0 if dirn == "mp" else 64
            nc.sync.dma_start(zt_part_d[row0:row0 + 64, :], zpart[:])
        nc.gpsimd.collective_compute(
            "AllReduce", ALU.add, replica_groups=[list(range(R))],
            ins=[zt_part_d[:].opt()], outs=[zt_full_d[:].opt()])
        zT = a_sb.tile([128, B], F32, name="zT")
        nc.sync.dma_start(zT[:], zt_full_d[:])

        fc1W = a_sb.tile([128, 64], F32, name="fc1W")
        nc.sync.dma_start(fc1W[:], dram["fc1_W"][:])
        fc1b = a_sb.tile([64, 1], F32, name="fc1b")
        nc.sync.dma_start(fc1b[:], dram["fc1_b"][:, None])
        fc2W = a_sb.tile([64, 1], F32, name="fc2W")
        nc.sync.dma_start(fc2W[:], dram["fc2_W"][:])
        fc2b = a_sb.tile([1, 1], F32, name="fc2b")
        nc.sync.dma_start(fc2b[:], dram["fc2_b"][:, None])

        ps = smallps.tile([128, 512], F32, name="small_ps")
        nc.tensor.matmul(ps[0:64, 0:B], fc1W[:], zT[:], start=True, stop=True)
        h1 = a_sb.tile([65, B], F32, name="h1")
        nc.scalar.activation(h1[0:64, :], ps[0:64, 0:B], AF.Relu, bias=fc1b[:])
        ps2 = smallps.tile([128, 512], F32, name="small_ps")
        nc.tensor.matmul(ps2[0:1, 0:B], fc2W[:], h1[0:64, :],
                         start=True, stop=True)
        osb = a_sb.tile([1, B], F32, name="osb")
        nc.scalar.activation(osb[:], ps2[0:1, 0:B], AF.Sigmoid, bias=fc2b[:])
        nc.sync.dma_start(out_d[:], osb[:])

        exp_pool.release()
        ops.release()
        s12ps.release()
        smallps.release()
        a_sb.release()
        xnf_pool.release()
        xT_pool.release()
        dpool.release()
        const.release()

    nc.compile()
    return nc



# ----------------------------------------------------------------- entry

def kernel(**inputs):
    global last_results
    meta, percore = _prep_host(inputs)
    key = (meta["mol_T_blk"], meta["prot_T_blk"])
    if key not in _CACHE:
        _CACHE[key] = _build(meta)
    nc = _CACHE[key]
    from concourse.bass_utils import run_bass_kernel_spmd
    res = run_bass_kernel_spmd(nc, percore, list(range(R)))
    last_results = res
    return np.asarray(res.results[0]["out"], np.float32).reshape(B)



# revision 19
# speedup vs baseline: 1.0371x; 1.0371x over previous
"""CrossGraphAttentionModel on 8 Trainium2 NeuronCores (Bass/Tile, SPMD).

Sharding: nodes/edges of both graphs are sharded 8 ways by (dst-sorted) node
range; 64-dim weights replicated, all in bf16 on the PE. Per GINE layer the
x[src] gather is done ON the PE: edges are sorted by (dst window, src block)
with runs padded to 64, so every 64-edge half-tile reads one 128-node block
of the AllGathered x; a host-built src-mod-128 one-hot (lhsT) contracts
against that block, whose index is loaded per half-tile from a per-core table
into PE registers (dynamic rhs offset). The edge-linear term accumulates into
the same PSUM via a second matmul, ACT applies the relu, and a dst one-hot
matmul scatter-adds messages per 128-node window. Cross-graph attention is
single-pass: scores use contraction-17 matmuls (K^T tiles with a ones row
against per-head Q with a folded Cauchy-Schwarz row bound), exp runs on ACT
into bf16, and the wV product accumulates transposed [17, Nq] outputs with a
ones column producing softmax denominators for free. Graph pooling is a
one-hot matmul with 1/count weights, AllReduced, then the tiny output MLP.
"""

import numpy as np
import ml_dtypes

BF = ml_dtypes.bfloat16

R = 8
HID = 64
B = 32
HEADS = 4
HD = 16
N_MOL, N_PROT = 2048, 4096
E_MOL, E_PROT = 32768, 131072
NC_MOL, NC_PROT = N_MOL // R, N_PROT // R          # 256, 512
NBLK_MOL, NBLK_PROT = NC_MOL // 128, NC_PROT // 128  # 2, 4
NBF_MOL, NBF_PROT = N_MOL // 128, N_PROT // 128      # 16, 32

_CACHE = {}
last_results = None


# ----------------------------------------------------------------- host prep

def _prep_edges(edge_index, eattr, nblk):
    """Sort edges by dst window, then by src block inside each window, with
    every (window, srcblk) run padded to a multiple of 64 edges. Produces the
    per-core one-hot operands and the per-half-tile source-block table."""
    src, dst = np.asarray(edge_index[0]), np.asarray(edge_index[1])
    eattr = np.asarray(eattr, np.float32)
    D = eattr.shape[1]
    nwin = R * nblk

    order = np.argsort(dst, kind="stable")
    src_s, dst_s, ea_s = src[order], dst[order], eattr[order]
    win = dst_s // 128
    counts = np.bincount(win, minlength=nwin)
    starts = np.concatenate([[0], np.cumsum(counts)])

    # per-window edge lists grouped by srcblk, runs padded to 64
    win_srcmod = []
    win_dstloc = []
    win_ea = []
    win_blk64 = []   # srcblk per 64-edge slot
    padded_len = np.zeros(nwin, np.int64)
    for g in range(nwin):
        lo, hi = starts[g], starts[g + 1]
        s_g, d_g, e_g = src_s[lo:hi], dst_s[lo:hi], ea_s[lo:hi]
        o2 = np.argsort(s_g // 128, kind="stable")
        s_g, d_g, e_g = s_g[o2], d_g[o2], e_g[o2]
        blk = s_g // 128
        sm_l, dl_l, ea_l, b64_l = [], [], [], []
        for bk in np.unique(blk):
            m = blk == bk
            n = int(m.sum())
            npad = -n % 64
            sm = np.concatenate([s_g[m] % 128, np.full(npad, -1, np.int64)])
            dl = np.concatenate([d_g[m] - g * 128, np.full(npad, -1, np.int64)])
            ea = np.concatenate([e_g[m], np.zeros((npad, D), np.float32)])
            sm_l.append(sm); dl_l.append(dl); ea_l.append(ea)
            b64_l.extend([int(bk)] * ((n + npad) // 64))
        win_srcmod.append(np.concatenate(sm_l) if sm_l else np.zeros(0, np.int64))
        win_dstloc.append(np.concatenate(dl_l) if dl_l else np.zeros(0, np.int64))
        win_ea.append(np.concatenate(ea_l) if ea_l else np.zeros((0, D), np.float32))
        win_blk64.append(b64_l)
        padded_len[g] = len(win_srcmod[-1])

    T_blk = max(1, int(np.ceil(padded_len.max() / 128)))
    T_total = nblk * T_blk
    E_core = T_total * 128

    Wea = ((T_total + 1) // 2) * 128
    ohmod = np.zeros((R, 128, E_core), BF)
    ohdst = np.zeros((R, 128, T_total, 128), BF)
    eaT = np.zeros((R, D + 1, E_core), np.float32)
    segblk = np.zeros((R, 1, 2 * T_total), np.int32)
    for c in range(R):
        for b in range(nblk):
            g = c * nblk + b
            sm, dl, ea = win_srcmod[g], win_dstloc[g], win_ea[g]
            n = len(sm)
            off = b * T_blk * 128
            e_idx = off + np.arange(n)
            real = sm >= 0
            ohmod[c, sm[real], e_idx[real]] = 1.0
            t_idx = e_idx // 128
            p_idx = e_idx % 128
            ohdst[c, p_idx[real], t_idx[real], dl[real]] = 1.0
            eaT[c, :D, off:off + n] = ea.T
            eaT[c, D, off:off + n] = real.astype(np.float32)
            b64 = win_blk64[g]
            h0 = (b * T_blk) * 2
            segblk[c, 0, h0:h0 + len(b64)] = b64
    # pack eaT 2-up: tile t -> rows 64*(t%2)..+11, cols (t//2)*128..+128
    eaT_w = np.zeros((R, 75, Wea), np.float32)
    for t in range(T_total):
        rb, cb = 64 * (t % 2), 128 * (t // 2)
        eaT_w[:, rb:rb + D + 1, cb:cb + 128] = \
            eaT[:, :, t * 128:(t + 1) * 128]
    return dict(T_blk=T_blk, T_total=T_total, E_core=E_core, D=D, Wea=Wea,
                ohmod=ohmod, ohdst=ohdst,
                eaT_w=np.ascontiguousarray(eaT_w.astype(BF)), segblk=segblk)


def _prep_host(inp):
    """All integer/layout preprocessing. Returns (meta, per_core_inputs)."""
    mol = _prep_edges(inp["mol_edge_index"], inp["mol_eattr"], NBLK_MOL)
    prot = _prep_edges(inp["prot_edge_index"], inp["prot_eattr"], NBLK_PROT)

    # pool matrices with 1/count entries, wrapped [128, nblk, B]
    def pmat(batch, ncore, nblk):
        batch = np.asarray(batch)
        cnt = np.bincount(batch, minlength=B).astype(np.float32)
        inv = 1.0 / np.maximum(cnt, 1.0)
        m = np.zeros((R, ncore, B), np.float32)
        for c in range(R):
            sl = batch[c * ncore:(c + 1) * ncore]
            m[c, np.arange(ncore), sl] = inv[sl]
        return np.ascontiguousarray(
            m.reshape(R, nblk, 128, B).transpose(0, 2, 1, 3))

    mol_pmat = pmat(inp["mol_batch"], NC_MOL, NBLK_MOL)
    prot_pmat = pmat(inp["prot_batch"], NC_PROT, NBLK_PROT)

    # node features transposed per core with ones row (fp32)
    def xt(x, ncore):
        x = np.asarray(x, np.float32)
        d = x.shape[1]
        out = np.zeros((R, d + 1, ncore), np.float32)
        for c in range(R):
            out[c, :d] = x[c * ncore:(c + 1) * ncore].T
            out[c, d] = 1.0
        return out

    mol_xT = xt(inp["mol_x"], NC_MOL)        # [R, 12, 256]
    prot_xT = xt(inp["prot_x"], NC_PROT)     # [R, 16, 512]

    ident_bf = np.eye(128, dtype=BF)
    ident_f32 = np.eye(128, dtype=np.float32)

    def cat_wb(W, b):  # -> [din+1, dout] fp32
        W = np.asarray(W, np.float32)
        b = np.asarray(b, np.float32)
        return np.concatenate([W, b[None, :]], 0)

    percore = []
    for c in range(R):
        m = {
            "mol_xT": mol_xT[c], "prot_xT": prot_xT[c],
            "mol_eaT": mol["eaT_w"][c], "prot_eaT": prot["eaT_w"][c],
            "mol_ohmod": mol["ohmod"][c], "prot_ohmod": prot["ohmod"][c],
            "mol_ohdst": mol["ohdst"][c], "prot_ohdst": prot["ohdst"][c],
            "mol_segblk": mol["segblk"][c], "prot_segblk": prot["segblk"][c],
            "mol_pmat": mol_pmat[c], "prot_pmat": prot_pmat[c],
            "ident_bf": ident_bf, "ident_f32": ident_f32,
            "node_lin_mol_W": cat_wb(inp["node_lin_mol_W"], inp["node_lin_mol_b"]),
            "node_lin_prot_W": cat_wb(inp["node_lin_prot_W"], inp["node_lin_prot_b"]),
            "edge_lin_mol_W": np.ascontiguousarray(np.tile(
                np.pad(cat_wb(inp["edge_lin_mol_W"], inp["edge_lin_mol_b"]),
                       ((0, 53), (0, 0))), (2, 1))[:75]).astype(BF),
            "edge_lin_prot_W": np.ascontiguousarray(np.tile(
                np.pad(cat_wb(inp["edge_lin_prot_W"], inp["edge_lin_prot_b"]),
                       ((0, 53), (0, 0))), (2, 1))[:75]).astype(BF),
            "fc1_W": np.asarray(inp["fc1_W"], np.float32),
            "fc1_b": np.asarray(inp["fc1_b"], np.float32),
            "fc2_W": np.asarray(inp["fc2_W"], np.float32),
            "fc2_b": np.asarray(inp["fc2_b"], np.float32),
        }
        def hilo(w):
            hi = w.astype(BF)
            lo = (w - hi.astype(np.float32)).astype(BF)
            return hi, lo
        for s in ("mol", "prot"):
            for l in range(3):
                for nm, wf in (("W1", "b1"), ("W2", "b2")):
                    w = cat_wb(inp[f"{s}_conv_{nm}"][l], inp[f"{s}_conv_{wf}"][l])
                    hi, lo = hilo(w)
                    m[f"{s}_conv_{nm}_{l}_hi"] = hi
                    m[f"{s}_conv_{nm}_{l}_lo"] = lo
        for d in ("mp", "pm"):
            W = np.asarray(inp[f"attn_{d}_W"], np.float32)
            bb = np.asarray(inp[f"attn_{d}_b"], np.float32)
            wq = cat_wb(W[0], bb[0]) * 0.25
            wv = cat_wb(W[2], bb[2])
            wk = cat_wb(W[1], bb[1])
            wka = np.zeros((65, 68), np.float32)
            for h in range(HEADS):
                wka[:, 17 * h:17 * h + 16] = wk[:, 16 * h:16 * h + 16]
                wka[64, 17 * h + 16] = 1.0
            for nm, w in (("q", wq), ("k", wka), ("v", wv)):
                hi, lo = hilo(w)
                m[f"attn_{d}_W{nm}_hi"] = hi
                m[f"attn_{d}_W{nm}_lo"] = lo
        percore.append(m)

    meta = dict(mol_T_blk=mol["T_blk"], mol_T_total=mol["T_total"],
                mol_E_core=mol["E_core"], mol_Wea=mol["Wea"],
                prot_T_blk=prot["T_blk"], prot_T_total=prot["T_total"],
                prot_E_core=prot["E_core"], prot_Wea=prot["Wea"])
    return meta, percore


# ------------------------------------------------------------- device build

def _build(meta):
    import concourse.bacc as bacc
    import concourse.mybir as mybir
    import concourse.tile as tile
    from concourse.bass import ds

    F32 = mybir.dt.float32
    BF16 = mybir.dt.bfloat16
    I32 = mybir.dt.int32
    AF = mybir.ActivationFunctionType
    ALU = mybir.AluOpType
    AX = mybir.AxisListType

    nc = bacc.Bacc("TRN2", target_bir_lowering=False, debug=False,
                   num_devices=R)

    dram = {}

    def din(name, shape, dtype=F32):
        dram[name] = nc.dram_tensor(name, list(shape), dtype,
                                    kind="ExternalInput")
        return dram[name]

    mT, mE = meta["mol_T_total"], meta["mol_E_core"]
    pT, pE = meta["prot_T_total"], meta["prot_E_core"]

    din("mol_xT", [12, NC_MOL]); din("prot_xT", [16, NC_PROT])
    din("mol_eaT", [75, meta["mol_Wea"]], BF16)
    din("prot_eaT", [75, meta["prot_Wea"]], BF16)
    din("mol_ohmod", [128, mE], BF16); din("prot_ohmod", [128, pE], BF16)
    din("mol_ohdst", [128, mT, 128], BF16)
    din("prot_ohdst", [128, pT, 128], BF16)
    din("mol_segblk", [1, 2 * mT], I32); din("prot_segblk", [1, 2 * pT], I32)
    din("mol_pmat", [128, NBLK_MOL, B])
    din("prot_pmat", [128, NBLK_PROT, B])
    din("ident_bf", [128, 128], BF16); din("ident_f32", [128, 128])
    din("node_lin_mol_W", [12, 64]); din("node_lin_prot_W", [16, 64])
    din("edge_lin_mol_W", [75, 64], BF16)
    din("edge_lin_prot_W", [75, 64], BF16)
    for s in ("mol", "prot"):
        for l in range(3):
            for nm in ("W1", "W2"):
                din(f"{s}_conv_{nm}_{l}_hi", [65, 64], BF16)
                din(f"{s}_conv_{nm}_{l}_lo", [65, 64], BF16)
    for d in ("mp", "pm"):
        for p in ("hi", "lo"):
            din(f"attn_{d}_Wq_{p}", [65, 64], BF16)
            din(f"attn_{d}_Wk_{p}", [65, 68], BF16)
            din(f"attn_{d}_Wv_{p}", [65, 64], BF16)
    din("fc1_W", [128, 64]); din("fc1_b", [64])
    din("fc2_W", [64, 1]); din("fc2_b", [1])

    out_d = nc.dram_tensor("out", [1, B], F32, kind="ExternalOutput")

    sides = {
        "mol": dict(N=N_MOL, NC=NC_MOL, nblk=NBLK_MOL, nbf=NBF_MOL,
                    T_blk=meta["mol_T_blk"], T_total=mT, E_core=mE, din=12),
        "prot": dict(N=N_PROT, NC=NC_PROT, nblk=NBLK_PROT, nbf=NBF_PROT,
                     T_blk=meta["prot_T_blk"], T_total=pT, E_core=pE, din=16),
    }

    with tile.TileContext(nc) as tc:
        const = tc.alloc_tile_pool(name="const", bufs=1)

        def load_const(name, shape, dtype=F32):
            t = const.tile(list(shape), dtype, name=f"c_{name}")
            nc.sync.dma_start(t[:], dram[name][:])
            return t

        ident_bf = load_const("ident_bf", [128, 128], BF16)
        ident_f32 = load_const("ident_f32", [128, 128])

        Wn = {"mol": load_const("node_lin_mol_W", [12, 64]),
              "prot": load_const("node_lin_prot_W", [16, 64])}
        W1 = {s: [[load_const(f"{s}_conv_W1_{l}_{p}", [65, 64], BF16)
                   for p in ("hi", "lo")] for l in range(3)] for s in sides}
        W2 = {s: [[load_const(f"{s}_conv_W2_{l}_{p}", [65, 64], BF16)
                   for p in ("hi", "lo")] for l in range(3)] for s in sides}
        sb_pmat = {s: load_const(f"{s}_pmat", [128, sides[s]["nblk"], B])
                   for s in sides}

        # ---------------- DRAM internals (bf16 node-major x)
        dpool = tc.alloc_tile_pool(name="dram", bufs=1, space="DRAM")
        x_sh_d = {s: [dpool.tile([sides[s]["NC"], 64 if l < 3 else 128], BF16,
                                 name=f"xsh_{s}_{l}") for l in range(4)]
                  for s in sides}
        x_full_d = {s: [dpool.tile([sides[s]["N"], 64 if l < 3 else 128], BF16,
                                   addr_space="Shared", name=f"xfull_{s}_{l}")
                        for l in range(4)] for s in sides}

        # ---------------- SBUF pools
        xT_pool = tc.alloc_tile_pool(name="xT", bufs=2)
        xnf_pool = tc.alloc_tile_pool(name="xnf", bufs=2)
        xfull_pool = tc.alloc_tile_pool(name="xfull", bufs=1)
        gmem = tc.alloc_tile_pool(name="gmem", bufs=1)
        msg_pool = tc.alloc_tile_pool(name="msg", bufs=3)
        gconst = tc.alloc_tile_pool(name="gconst", bufs=1)

        def load_gconst(name, shape, dtype=F32):
            t = gconst.tile(list(shape), dtype, name=f"g_{name}")
            nc.sync.dma_start(t[:], dram[name][:])
            return t

        We = {"mol": load_gconst("edge_lin_mol_W", [75, 64], BF16),
              "prot": load_gconst("edge_lin_prot_W", [75, 64], BF16)}
        sb_xTin = {"mol": load_gconst("mol_xT", [12, NC_MOL]),
                   "prot": load_gconst("prot_xT", [16, NC_PROT])}
        sb_seg = {s: load_gconst(f"{s}_segblk", [1, 2 * sides[s]["T_total"]],
                                 I32) for s in sides}
        sb_eaT, sb_ohmod, sb_ohdst = {}, {}, {}
        for s in sides:
            sd = sides[s]
            sb_eaT[s] = load_gconst(f"{s}_eaT", [75, meta[f"{s}_Wea"]], BF16)
            sb_ohmod[s] = load_gconst(f"{s}_ohmod", [128, sd["E_core"]], BF16)
            sb_ohdst[s] = load_gconst(f"{s}_ohdst",
                                      [128, sd["T_total"], 128], BF16)

        msgps = tc.alloc_tile_pool(name="msgps", bufs=2, space="PSUM")
        aggps = tc.alloc_tile_pool(name="aggps", bufs=2, space="PSUM")
        mlpps = tc.alloc_tile_pool(name="mlpps", bufs=2, space="PSUM")
        trps = tc.alloc_tile_pool(name="trps", bufs=2, space="PSUM")

        def xT_to_xnf_and_gather(s, l, xThi, xTlo=None):
            """Transpose xT shard to node-major, publish shard, AllGather.
            Last layer publishes hi|lo side by side in 128 feature cols."""
            sd = sides[s]
            nblk = sd["nblk"]
            fw = 64 if xTlo is None else 128
            xnf = xnf_pool.tile([128, nblk, fw], BF16, name=f"xnf_{s}_{fw}",
                                tag=f"xnf_{s}")
            for b in range(nblk):
                tp = trps.tile([128, 64], BF16, name="tr_ps")
                nc.tensor.transpose(tp[:], xThi[0:64, b * 128:(b + 1) * 128],
                                    ident_bf[0:64, 0:64])
                nc.vector.tensor_copy(xnf[:, b, 0:64], tp[:])
                if xTlo is not None:
                    tp2 = trps.tile([128, 64], BF16, name="tr_ps")
                    nc.tensor.transpose(tp2[:],
                                        xTlo[0:64, b * 128:(b + 1) * 128],
                                        ident_bf[0:64, 0:64])
                    nc.vector.tensor_copy(xnf[:, b, 64:128], tp2[:])
            nc.sync.dma_start(
                x_sh_d[s][l][:].rearrange("(t p) f -> p t f", p=128), xnf[:])
            nc.gpsimd.collective_compute(
                "AllGather", ALU.bypass, replica_groups=[list(range(R))],
                ins=[x_sh_d[s][l][:].opt()], outs=[x_full_d[s][l][:].opt()])
            return xnf

        def load_xfull(s, l):
            sd = sides[s]
            fw = 64 if l < 3 else 128
            xf = xfull_pool.tile([128, sd["nbf"], fw], BF16,
                                 name=f"xf_{s}_{fw}", tag=f"xf_{s}")
            nc.sync.dma_start(
                xf[:], x_full_d[s][l][:].rearrange("(t p) f -> p t f", p=128))
            return xf

        # initial node features x0 (no relu); x kept as f32 + bf16 hi/lo
        def make_triple(s, ps_ap, NCs, act):
            xTf = xT_pool.tile([65, NCs], F32, name=f"xTf_{s}", tag=f"xTf_{s}")
            nc.scalar.activation(xTf[0:64, :], ps_ap, act)
            xThi = xT_pool.tile([65, NCs], BF16, name=f"xTh_{s}",
                                tag=f"xTh_{s}")
            nc.scalar.activation(xThi[0:64, :], xTf[0:64, :], AF.Copy)
            xTlo = xT_pool.tile([65, NCs], BF16, name=f"xTl_{s}",
                                tag=f"xTl_{s}")
            nc.vector.tensor_sub(xTlo[0:64, :], xTf[0:64, :], xThi[0:64, :])
            nc.vector.memset(xThi[64:65, :], 1.0)
            nc.vector.memset(xTlo[64:65, :], 0.0)
            return xTf, xThi, xTlo

        xT_cur = {}
        xnf_cur = {}
        for s in sides:
            sd = sides[s]
            NCs = sd["NC"]
            ps = mlpps.tile([64, 512], F32, name="mlp_ps")
            nc.tensor.matmul(ps[:, 0:NCs], Wn[s][:], sb_xTin[s][:],
                             start=True, stop=True)
            xT_cur[s] = make_triple(s, ps[:, 0:NCs], NCs, AF.Copy)
            xnf_cur[s] = xT_to_xnf_and_gather(s, 0, xT_cur[s][1])

        # ---------------- GINE layers
        segregs = [nc.tensor.alloc_register(f"segreg{i}") for i in range(8)]
        seg_cnt = [0]

        def seg_val(ap, hi):
            r = segregs[seg_cnt[0] % 8]
            seg_cnt[0] += 1
            nc.tensor.reg_load(r, ap)
            return nc.tensor.snap(r, donate=True, min_val=0, max_val=hi)

        for l in range(3):
            for s in ("prot", "mol"):
                sd = sides[s]
                NCs, nblk, T_blk, nbf = sd["NC"], sd["nblk"], sd["T_blk"], sd["nbf"]
                xfull = load_xfull(s, l)
                xTf_prev = xT_cur[s][0]
                hTf = gmem.tile([65, NCs], F32, name=f"hTf_{s}",
                                tag=f"hTf_{s}")
                for b in range(nblk):
                    agg = aggps.tile([64, 128], F32, name="agg_ps")
                    for g0 in range(0, T_blk, 8):
                        ng = min(8, T_blk - g0)
                        mps = msgps.tile([128, 8, 64], F32, name="msg_ps")
                        for j in range(ng):
                            t = b * T_blk + g0 + j
                            e0 = t * 128
                            vA = seg_val(sb_seg[s][0:1, 2 * t:2 * t + 1],
                                         nbf - 1)
                            nc.tensor.matmul(
                                mps[0:64, j, :],
                                sb_ohmod[s][:, e0:e0 + 64],
                                xfull[:, ds(vA, 1), :],
                                start=True, stop=False, tile_position=(0, 0),
                                skip_group_check=True)
                            vB = seg_val(sb_seg[s][0:1, 2 * t + 1:2 * t + 2],
                                         nbf - 1)
                            nc.tensor.matmul(
                                mps[64:128, j, :],
                                sb_ohmod[s][:, e0 + 64:e0 + 128],
                                xfull[:, ds(vB, 1), :],
                                start=True, stop=False, tile_position=(0, 64),
                                skip_group_check=True)
                            rb, cb = 64 * (t % 2), 128 * (t // 2)
                            nc.tensor.matmul(
                                mps[:, j, :],
                                sb_eaT[s][rb:rb + 11, cb:cb + 128],
                                We[s][rb:rb + 11, :], start=False, stop=True,
                                skip_group_check=True)
                        msg = msg_pool.tile([128, 8, 64], BF16, name="msg_sb")
                        nc.scalar.activation(msg[:, 0:ng, :], mps[:, 0:ng, :],
                                             AF.Relu)
                        for j in range(ng):
                            t = b * T_blk + g0 + j
                            nc.tensor.matmul(
                                agg[:], msg[:, j, :], sb_ohdst[s][:, t, :],
                                start=(g0 + j == 0),
                                stop=(g0 + j == T_blk - 1),
                                skip_group_check=True)
                    nc.vector.tensor_add(hTf[0:64, b * 128:(b + 1) * 128],
                                         xTf_prev[0:64, b * 128:(b + 1) * 128],
                                         agg[:])
                hThi = gmem.tile([65, NCs], BF16, name=f"hTh_{s}",
                                 tag=f"hTh_{s}")
                nc.scalar.activation(hThi[0:64, :], hTf[0:64, :], AF.Copy)
                hTlo = gmem.tile([65, NCs], BF16, name=f"hTl_{s}",
                                 tag=f"hTl_{s}")
                nc.vector.tensor_sub(hTlo[0:64, :], hTf[0:64, :],
                                     hThi[0:64, :])
                nc.vector.memset(hThi[64:65, :], 1.0)
                nc.vector.memset(hTlo[64:65, :], 0.0)

                def mlp3(Wp, rhs_hi, rhs_lo, NCs):
                    ps_ = mlpps.tile([64, 512], F32, name="mlp_ps")
                    nc.tensor.matmul(ps_[:, 0:NCs], Wp[0][:], rhs_hi[:],
                                     start=True, stop=False,
                                     skip_group_check=True)
                    nc.tensor.matmul(ps_[:, 0:NCs], Wp[0][:], rhs_lo[:],
                                     start=False, stop=False,
                                     skip_group_check=True)
                    nc.tensor.matmul(ps_[:, 0:NCs], Wp[1][:], rhs_hi[:],
                                     start=False, stop=True,
                                     skip_group_check=True)
                    return ps_

                ps1 = mlp3(W1[s][l], hThi, hTlo, NCs)
                r1f = gmem.tile([65, NCs], F32, name=f"r1f_{s}",
                                tag=f"r1f_{s}")
                nc.scalar.activation(r1f[0:64, :], ps1[:, 0:NCs], AF.Relu)
                r1hi = gmem.tile([65, NCs], BF16, name=f"r1h_{s}",
                                 tag=f"r1h_{s}")
                nc.scalar.activation(r1hi[0:64, :], r1f[0:64, :], AF.Copy)
                r1lo = gmem.tile([65, NCs], BF16, name=f"r1l_{s}",
                                 tag=f"r1l_{s}")
                nc.vector.tensor_sub(r1lo[0:64, :], r1f[0:64, :],
                                     r1hi[0:64, :])
                nc.vector.memset(r1hi[64:65, :], 1.0)
                nc.vector.memset(r1lo[64:65, :], 0.0)
                ps2 = mlp3(W2[s][l], r1hi, r1lo, NCs)
                xT_cur[s] = make_triple(s, ps2[:, 0:NCs], NCs, AF.Relu)
                xnf_cur[s] = xT_to_xnf_and_gather(
                    s, l + 1, xT_cur[s][1],
                    xT_cur[s][2] if l == 2 else None)

        for p in (trps, mlpps, aggps, msgps):
            p.release()
        gconst.release()
        msg_pool.release()
        gmem.release()

        # ---------------- attention phase
        a_sb = tc.alloc_tile_pool(name="attn_sb", bufs=1)
        smallps = tc.alloc_tile_pool(name="smallps", bufs=2, space="PSUM")
        s12ps = tc.alloc_tile_pool(name="s12ps", bufs=2, space="PSUM")
        ops = tc.alloc_tile_pool(name="ops", bufs=1, space="PSUM")
        ex_pool = tc.alloc_tile_pool(name="expt", bufs=6)

        def sps():
            return smallps.tile([128, 512], F32, name="small_ps")

        def sbf():
            return smallps.tile([128, 128], BF16, name="small_bf")

        # final x of both sides: load node-major hi|lo, build transposed pair
        xT_full = {}
        for s in sides:
            sd = sides[s]
            Ns, nbf = sd["N"], sd["nbf"]
            xf = load_xfull(s, 3)
            xT_fh = a_sb.tile([65, Ns], BF16, name=f"xTfullh_{s}")
            xT_fl = a_sb.tile([65, Ns], BF16, name=f"xTfulll_{s}")
            for t in range(nbf):
                tp = sbf()
                nc.tensor.transpose(tp[0:64, 0:128], xf[:, t, 0:64],
                                    ident_bf[:])
                nc.vector.tensor_copy(xT_fh[0:64, t * 128:(t + 1) * 128],
                                      tp[0:64, 0:128])
                tp2 = sbf()
                nc.tensor.transpose(tp2[0:64, 0:128], xf[:, t, 64:128],
                                    ident_bf[:])
                nc.vector.tensor_copy(xT_fl[0:64, t * 128:(t + 1) * 128],
                                      tp2[0:64, 0:128])
            nc.vector.memset(xT_fh[64:65, :], 1.0)
            nc.vector.memset(xT_fl[64:65, :], 0.0)
            xT_full[s] = (xT_fh, xT_fl)

        H_sb = {}
        for dirn, (qs, ks) in (("mp", ("mol", "prot")), ("pm", ("prot", "mol"))):
            qd, kd = sides[qs], sides[ks]
            NCq, Nk = qd["NC"], kd["N"]
            n_qt = NCq // 128
            n_k128 = Nk // 128
            n_k512 = Nk // 512
            Wq, Wk, Wv = [], [], []
            for p, lst, wd in (("hi", Wq, 64), ("lo", Wq, 64),
                               ("hi", Wk, 68), ("lo", Wk, 68),
                               ("hi", Wv, 64), ("lo", Wv, 64)):
                nm = "q" if lst is Wq else ("k" if lst is Wk else "v")
                t = a_sb.tile([65, wd], BF16, name=f"W{nm}{p}_{dirn}")
                nc.sync.dma_start(t[:], dram[f"attn_{dirn}_W{nm}_{p}"][:])
                lst.append(t)

            def mm3w(out_ap, Wpair, csl, xpair, rsl, NCo):
                nc.tensor.matmul(out_ap, Wpair[0][:, csl], xpair[0][:, rsl],
                                 start=True, stop=False,
                                 skip_group_check=True)
                nc.tensor.matmul(out_ap, Wpair[0][:, csl], xpair[1][:, rsl],
                                 start=False, stop=False,
                                 skip_group_check=True)
                nc.tensor.matmul(out_ap, Wpair[1][:, csl], xpair[0][:, rsl],
                                 start=False, stop=True,
                                 skip_group_check=True)

            # K^T per head with ones row (via augmented Wk), hi/lo pair
            KTh = [a_sb.tile([81, Nk], BF16, name=f"KTh_{dirn}_{i}")
                   for i in range(2)]
            KTl = [a_sb.tile([81, Nk], BF16, name=f"KTl_{dirn}_{i}")
                   for i in range(2)]
            for h in range(HEADS):
                r0 = 64 * (h % 2)
                for cc in range(n_k512):
                    csl = slice(cc * 512, (cc + 1) * 512)
                    pk = sps()[r0:r0 + 17, :]
                    mm3w(pk[:], Wk, slice(17 * h, 17 * h + 17),
                         xT_full[ks], csl, 512)
                    nc.scalar.activation(KTh[h // 2][r0:r0 + 17, csl],
                                         pk[:], AF.Copy)
                    nc.vector.tensor_sub(KTl[h // 2][r0:r0 + 17, csl],
                                         pk[:], KTh[h // 2][r0:r0 + 17, csl])

            # Q^T per head (0.25 folded in Wq); hi has the -m row, plus lo
            QTo = [a_sb.tile([81, NCq], BF16, name=f"QTo_{dirn}_{i}")
                   for i in range(2)]
            QTl = [a_sb.tile([81, NCq], BF16, name=f"QTl_{dirn}_{i}")
                   for i in range(2)]
            for h in range(HEADS):
                r0 = 64 * (h % 2)
                pq = sps()[r0:r0 + 16, :]
                mm3w(pq[:, 0:NCq], Wq, slice(16 * h, 16 * h + 16),
                     (xT_cur[qs][1], xT_cur[qs][2]), slice(0, NCq), NCq)
                nc.scalar.activation(QTo[h // 2][r0:r0 + 16, :], pq[:, 0:NCq],
                                     AF.Copy)
                nc.vector.tensor_sub(QTl[h // 2][r0:r0 + 16, :], pq[:, 0:NCq],
                                     QTo[h // 2][r0:r0 + 16, :])
            for h in range(HEADS):
                QTt, KTt, r0 = QTo[h // 2], KTh[h // 2], 64 * (h % 2)
                negm = a_sb.tile([1, NCq], BF16, name="negm", bufs=2,
                                 tag="negm")
                for qt in range(n_qt):
                    mx = a_sb.tile([128, n_k512], F32, name="mx", bufs=2,
                                   tag="mx")
                    for cch in range(n_k512):
                        spT = s12ps.tile([128, 512], F32, name="s_ps")
                        nc.tensor.matmul(
                            spT[:],
                            QTt[r0:r0 + 16, qt * 128:(qt + 1) * 128],
                            KTt[r0:r0 + 16, cch * 512:(cch + 1) * 512],
                            start=True, stop=True)
                        nc.vector.reduce_max(mx[:, cch:cch + 1], spT[:],
                                             axis=AX.X)
                    mqt = a_sb.tile([128, 1], F32, name="mqt", bufs=2,
                                    tag="mqt")
                    nc.vector.reduce_max(mqt[:], mx[:], axis=AX.X)
                    tpm = sps()[0:1, 0:128]
                    nc.tensor.transpose(tpm[:], mqt[:], ident_f32[:])
                    nc.scalar.activation(negm[0:1, qt * 128:(qt + 1) * 128],
                                         tpm[:], AF.Copy, scale=-1.0)
                nc.sync.dma_start(QTt[r0 + 16:r0 + 17, :], negm[:])

            # V' [128, n_k128, 4, 34]: cols 0-15 Vhi, 16 ones, 17-32 Vlo,
            # 33 zero; hi and lo halves feed two accumulating wV matmuls
            Vp = a_sb.tile([128, n_k128, HEADS, 34], BF16, name=f"Vp_{dirn}")
            nc.vector.memset(Vp[:, :, :, 16:17], 1.0)
            nc.vector.memset(Vp[:, :, :, 33:34], 0.0)
            for kt in range(n_k128):
                ksl = slice(kt * 128, (kt + 1) * 128)
                pv = sps()[:, 0:64]
                nc.tensor.matmul(pv[:], xT_full[ks][0][:, ksl], Wv[0][:],
                                 start=True, stop=False,
                                 skip_group_check=True)
                nc.tensor.matmul(pv[:], xT_full[ks][1][:, ksl], Wv[0][:],
                                 start=False, stop=False,
                                 skip_group_check=True)
                nc.tensor.matmul(pv[:], xT_full[ks][0][:, ksl], Wv[1][:],
                                 start=False, stop=True,
                                 skip_group_check=True)
                nc.scalar.activation(
                    Vp[:, kt, :, 0:16],
                    pv[:].rearrange("p (h d) -> p h d", h=HEADS), AF.Copy)
                nc.vector.tensor_sub(
                    Vp[:, kt, :, 17:33],
                    pv[:].rearrange("p (h d) -> p h d", h=HEADS),
                    Vp[:, kt, :, 0:16])

            # scores (hi.hi + hi.lo + lo.hi) -> exp -> wV, single pass
            o_ps = [ops.tile([81, NCq], F32, name=f"o_ps_{i}")
                    for i in range(2)]
            for kc in range(n_k128):
                ksl = slice(kc * 128, (kc + 1) * 128)
                for h in range(HEADS):
                    i2, r0 = h // 2, 64 * (h % 2)
                    sp = s12ps.tile([128, 512], F32, name="s_ps")[:, 0:NCq]
                    nc.tensor.matmul(sp[:], KTh[i2][r0:r0 + 17, ksl],
                                     QTo[i2][r0:r0 + 17, :],
                                     start=True, stop=False,
                                     skip_group_check=True)
                    nc.tensor.matmul(sp[:], KTh[i2][r0:r0 + 16, ksl],
                                     QTl[i2][r0:r0 + 16, :],
                                     start=False, stop=False,
                                     skip_group_check=True)
                    nc.tensor.matmul(sp[:], KTl[i2][r0:r0 + 16, ksl],
                                     QTo[i2][r0:r0 + 16, :],
                                     start=False, stop=True,
                                     skip_group_check=True)
                    ex = ex_pool.tile([128, NCq], BF16, name="ex",
                                      tag=f"ex_{dirn}")
                    nc.scalar.activation(ex[:], sp[:], AF.Exp)
                    nc.tensor.matmul(o_ps[i2][r0:r0 + 17, :],
                                     Vp[:, kc, h, 0:17], ex[:],
                                     start=(kc == 0), stop=False,
                                     skip_group_check=True)
                    nc.tensor.matmul(o_ps[i2][r0:r0 + 17, :],
                                     Vp[:, kc, h, 17:34], ex[:],
                                     start=False, stop=(kc == n_k128 - 1),
                                     skip_group_check=True)

            # normalize + assemble H (node-major, f32) + residual hi+lo
            H = a_sb.tile([128, n_qt, 64], F32, name=f"H_{dirn}")
            for h in range(HEADS):
                ro = 64 * (h % 2)
                osb = a_sb.tile([81, NCq], F32, name="osb", bufs=2, tag="osb")
                nc.vector.tensor_copy(osb[ro:ro + 17, :],
                                      o_ps[h // 2][ro:ro + 17, :])
                for qt in range(n_qt):
                    tp = sps()[:, 0:17]
                    nc.tensor.transpose(tp[:],
                                        osb[ro:ro + 17,
                                            qt * 128:(qt + 1) * 128],
                                        ident_f32[ro:ro + 17, ro:ro + 17])
                    inv1 = a_sb.tile([128, 1], F32, name="inv1", bufs=2,
                                     tag="inv1")
                    nc.vector.reciprocal(inv1[:], tp[:, 16:17])
                    nc.vector.tensor_scalar_mul(
                        H[:, qt, 16 * h:16 * (h + 1)], tp[:, 0:16], inv1[:])
            nc.vector.tensor_add(H[:], H[:], xnf_cur[qs][:, :, 0:64])
            nc.vector.tensor_add(H[:], H[:], xnf_cur[qs][:, :, 64:128])
            H_sb[dirn] = H

        # ---------------- pooling + output MLP
        zt_part_d = dpool.tile([128, B], F32, name="zt_part")
        zt_full_d = dpool.tile([128, B], F32, addr_space="Shared",
                               name="zt_full")
        for dirn, qs in (("mp", "mol"), ("pm", "prot")):
            n_qt = sides[qs]["NC"] // 128
            psz = sps()[0:64, 0:B]
            for qt in range(n_qt):
                nc.tensor.matmul(psz[:], H_sb[dirn][:, qt, :],
                                 sb_pmat[qs][:, qt, :],
                                 start=(qt == 0), stop=(qt == n_qt - 1),
                                 skip_group_check=True)
            zpart = a_sb.tile([64, B], F32, name=f"zpart_{dirn}")
            nc.vector.tensor_copy(zpart[:], psz[:])
            row0 = 0 if dirn == "mp" else 64
            nc.sync.dma_start(zt_part_d[row0:row0 + 64, :], zpart[:])
        nc.gpsimd.collective_compute(
            "AllReduce", ALU.add, replica_groups=[list(range(R))],
            ins=[zt_part_d[:].opt()], outs=[zt_full_d[:].opt()])
        zT = a_sb.tile([128, B], F32, name="zT")
        nc.sync.dma_start(zT[:], zt_full_d[:])

        fc1W = a_sb.tile([128, 64], F32, name="fc1W")
        nc.sync.dma_start(fc1W[:], dram["fc1_W"][:])
        fc1b = a_sb.tile([64, 1], F32, name="fc1b")
        nc.sync.dma_start(fc1b[:], dram["fc1_b"][:, None])
        fc2W = a_sb.tile([64, 1], F32, name="fc2W")
        nc.sync.dma_start(fc2W[:], dram["fc2_W"][:])
        fc2b = a_sb.tile([1, 1], F32, name="fc2b")
        nc.sync.dma_start(fc2b[:], dram["fc2_b"][:, None])

        ps = sps()[0:64, 0:B]
        nc.tensor.matmul(ps[:], fc1W[:], zT[:], start=True, stop=True)
        h1 = a_sb.tile([65, B], F32, name="h1")
        nc.scalar.activation(h1[0:64, :], ps[:], AF.Relu, bias=fc1b[:])
        ps2 = sps()[0:1, 0:B]
        nc.tensor.matmul(ps2[:], fc2W[:], h1[0:64, :], start=True, stop=True)
        osb = a_sb.tile([1, B], F32, name="osb_out")
        nc.scalar.activation(osb[:], ps2[:], AF.Sigmoid, bias=fc2b[:])
        nc.sync.dma_start(out_d[:], osb[:])

        ex_pool.release()
        ops.release()
        s12ps.release()
        smallps.release()
        a_sb.release()
        xfull_pool.release()
        xnf_pool.release()
        xT_pool.release()
        dpool.release()
        const.release()

    nc.compile()
    return nc


# ----------------------------------------------------------------- entry

def kernel(**inputs):
    global last_results
    meta, percore = _prep_host(inputs)
    key = (meta["mol_T_blk"], meta["prot_T_blk"])
    if key not in _CACHE:
        _CACHE[key] = _build(meta)
    nc = _CACHE[key]
    from concourse.bass_utils import run_bass_kernel_spmd
    res = run_bass_kernel_spmd(nc, percore, list(range(R)))
    last_results = res
    return np.asarray(res.results[0]["out"], np.float32).reshape(B)


# revision 20
# speedup vs baseline: 1.2442x; 1.1997x over previous
"""CrossGraphAttentionModel on 8 Trainium2 NeuronCores (Bass/Tile, SPMD).

Sharding: nodes/edges of both graphs are sharded 8 ways by (dst-sorted) node
range; 64-dim weights replicated, all in bf16 on the PE. Per GINE layer the
x[src] gather is done ON the PE: edges are sorted by (dst window, src block)
with runs padded to 64, so every 64-edge half-tile reads one 128-node block
of the AllGathered x; a host-built src-mod-128 one-hot (lhsT) contracts
against that block, whose index is loaded per half-tile from a per-core table
into PE registers (dynamic rhs offset). The edge-linear term accumulates into
the same PSUM via a second matmul, ACT applies the relu, and a dst one-hot
matmul scatter-adds messages per 128-node window. Cross-graph attention is
single-pass: scores use contraction-17 matmuls (K^T tiles with a ones row
against per-head Q with a folded Cauchy-Schwarz row bound), exp runs on ACT
into bf16, and the wV product accumulates transposed [17, Nq] outputs with a
ones column producing softmax denominators for free. Graph pooling is a
one-hot matmul with 1/count weights, AllReduced, then the tiny output MLP.
"""

import numpy as np
import ml_dtypes

BF = ml_dtypes.bfloat16

R = 8
HID = 64
B = 32
HEADS = 4
HD = 16
N_MOL, N_PROT = 2048, 4096
E_MOL, E_PROT = 32768, 131072
NC_MOL, NC_PROT = N_MOL // R, N_PROT // R          # 256, 512
NBLK_MOL, NBLK_PROT = NC_MOL // 128, NC_PROT // 128  # 2, 4
NBF_MOL, NBF_PROT = N_MOL // 128, N_PROT // 128      # 16, 32

_CACHE = {}
last_results = None


# ----------------------------------------------------------------- host prep

def _prep_edges(edge_index, eattr, nblk):
    """Sort edges by dst window, then by src block inside each window, with
    every (window, srcblk) run padded to a multiple of 64 edges. Produces the
    per-core one-hot operands and the per-half-tile source-block table."""
    src, dst = np.asarray(edge_index[0]), np.asarray(edge_index[1])
    eattr = np.asarray(eattr, np.float32)
    D = eattr.shape[1]
    nwin = R * nblk

    order = np.argsort(dst, kind="stable")
    src_s, dst_s, ea_s = src[order], dst[order], eattr[order]
    win = dst_s // 128
    counts = np.bincount(win, minlength=nwin)
    starts = np.concatenate([[0], np.cumsum(counts)])

    # per-window edge lists grouped by srcblk, runs padded to 64
    win_srcmod = []
    win_dstloc = []
    win_ea = []
    win_blk64 = []   # srcblk per 64-edge slot
    padded_len = np.zeros(nwin, np.int64)
    for g in range(nwin):
        lo, hi = starts[g], starts[g + 1]
        s_g, d_g, e_g = src_s[lo:hi], dst_s[lo:hi], ea_s[lo:hi]
        o2 = np.argsort(s_g // 128, kind="stable")
        s_g, d_g, e_g = s_g[o2], d_g[o2], e_g[o2]
        blk = s_g // 128
        sm_l, dl_l, ea_l, b64_l = [], [], [], []
        for bk in np.unique(blk):
            m = blk == bk
            n = int(m.sum())
            npad = -n % 64
            sm = np.concatenate([s_g[m] % 128, np.full(npad, -1, np.int64)])
            dl = np.concatenate([d_g[m] - g * 128, np.full(npad, -1, np.int64)])
            ea = np.concatenate([e_g[m], np.zeros((npad, D), np.float32)])
            sm_l.append(sm); dl_l.append(dl); ea_l.append(ea)
            b64_l.extend([int(bk)] * ((n + npad) // 64))
        win_srcmod.append(np.concatenate(sm_l) if sm_l else np.zeros(0, np.int64))
        win_dstloc.append(np.concatenate(dl_l) if dl_l else np.zeros(0, np.int64))
        win_ea.append(np.concatenate(ea_l) if ea_l else np.zeros((0, D), np.float32))
        win_blk64.append(b64_l)
        padded_len[g] = len(win_srcmod[-1])

    T_blk = max(1, int(np.ceil(padded_len.max() / 128)))
    T_total = nblk * T_blk
    E_core = T_total * 128

    Wea = ((T_total + 1) // 2) * 128
    ohmod = np.zeros((R, 128, E_core), BF)
    ohdst = np.zeros((R, 128, T_total, 128), BF)
    eaT = np.zeros((R, D + 1, E_core), np.float32)
    segblk = np.zeros((R, 1, 2 * T_total), np.int32)
    for c in range(R):
        for b in range(nblk):
            g = c * nblk + b
            sm, dl, ea = win_srcmod[g], win_dstloc[g], win_ea[g]
            n = len(sm)
            off = b * T_blk * 128
            e_idx = off + np.arange(n)
            real = sm >= 0
            ohmod[c, sm[real], e_idx[real]] = 1.0
            t_idx = e_idx // 128
            p_idx = e_idx % 128
            ohdst[c, p_idx[real], t_idx[real], dl[real]] = 1.0
            eaT[c, :D, off:off + n] = ea.T
            eaT[c, D, off:off + n] = real.astype(np.float32)
            b64 = win_blk64[g]
            h0 = (b * T_blk) * 2
            segblk[c, 0, h0:h0 + len(b64)] = b64
    # pack eaT 2-up: tile t -> rows 64*(t%2)..+11, cols (t//2)*128..+128
    eaT_w = np.zeros((R, 75, Wea), np.float32)
    for t in range(T_total):
        rb, cb = 64 * (t % 2), 128 * (t // 2)
        eaT_w[:, rb:rb + D + 1, cb:cb + 128] = \
            eaT[:, :, t * 128:(t + 1) * 128]
    return dict(T_blk=T_blk, T_total=T_total, E_core=E_core, D=D, Wea=Wea,
                ohmod=ohmod, ohdst=ohdst,
                eaT_w=np.ascontiguousarray(eaT_w.astype(BF)), segblk=segblk)


def _prep_host(inp):
    """All integer/layout preprocessing. Returns (meta, per_core_inputs)."""
    mol = _prep_edges(inp["mol_edge_index"], inp["mol_eattr"], NBLK_MOL)
    prot = _prep_edges(inp["prot_edge_index"], inp["prot_eattr"], NBLK_PROT)

    # pool matrices with 1/count entries, wrapped [128, nblk, B]
    def pmat(batch, ncore, nblk):
        batch = np.asarray(batch)
        cnt = np.bincount(batch, minlength=B).astype(np.float32)
        inv = 1.0 / np.maximum(cnt, 1.0)
        m = np.zeros((R, ncore, B), np.float32)
        for c in range(R):
            sl = batch[c * ncore:(c + 1) * ncore]
            m[c, np.arange(ncore), sl] = inv[sl]
        return np.ascontiguousarray(
            m.reshape(R, nblk, 128, B).transpose(0, 2, 1, 3))

    mol_pmat = pmat(inp["mol_batch"], NC_MOL, NBLK_MOL)
    prot_pmat = pmat(inp["prot_batch"], NC_PROT, NBLK_PROT)

    # node features transposed per core with ones row (fp32)
    def xt(x, ncore):
        x = np.asarray(x, np.float32)
        d = x.shape[1]
        out = np.zeros((R, d + 1, ncore), np.float32)
        for c in range(R):
            out[c, :d] = x[c * ncore:(c + 1) * ncore].T
            out[c, d] = 1.0
        return out

    mol_xT = xt(inp["mol_x"], NC_MOL)        # [R, 12, 256]
    prot_xT = xt(inp["prot_x"], NC_PROT)     # [R, 16, 512]

    ident_bf = np.eye(128, dtype=BF)
    ident_f32 = np.eye(128, dtype=np.float32)

    def cat_wb(W, b):  # -> [din+1, dout] fp32
        W = np.asarray(W, np.float32)
        b = np.asarray(b, np.float32)
        return np.concatenate([W, b[None, :]], 0)

    percore = []
    for c in range(R):
        m = {
            "mol_xT": mol_xT[c], "prot_xT": prot_xT[c],
            "mol_eaT": mol["eaT_w"][c], "prot_eaT": prot["eaT_w"][c],
            "mol_ohmod": mol["ohmod"][c], "prot_ohmod": prot["ohmod"][c],
            "mol_ohdst": mol["ohdst"][c], "prot_ohdst": prot["ohdst"][c],
            "mol_segblk": mol["segblk"][c], "prot_segblk": prot["segblk"][c],
            "mol_pmat": mol_pmat[c], "prot_pmat": prot_pmat[c],
            "ident_bf": ident_bf, "ident_f32": ident_f32,
            "node_lin_mol_W": cat_wb(inp["node_lin_mol_W"], inp["node_lin_mol_b"]),
            "node_lin_prot_W": cat_wb(inp["node_lin_prot_W"], inp["node_lin_prot_b"]),
            "edge_lin_mol_W": np.ascontiguousarray(np.tile(
                np.pad(cat_wb(inp["edge_lin_mol_W"], inp["edge_lin_mol_b"]),
                       ((0, 53), (0, 0))), (2, 1))[:75]).astype(BF),
            "edge_lin_prot_W": np.ascontiguousarray(np.tile(
                np.pad(cat_wb(inp["edge_lin_prot_W"], inp["edge_lin_prot_b"]),
                       ((0, 53), (0, 0))), (2, 1))[:75]).astype(BF),
            "fc1_W": np.asarray(inp["fc1_W"], np.float32),
            "fc1_b": np.asarray(inp["fc1_b"], np.float32),
            "fc2_W": np.asarray(inp["fc2_W"], np.float32),
            "fc2_b": np.asarray(inp["fc2_b"], np.float32),
        }
        def hilo(w):
            hi = w.astype(BF)
            lo = (w - hi.astype(np.float32)).astype(BF)
            return hi, lo
        for s in ("mol", "prot"):
            for l in range(3):
                for nm, wf in (("W1", "b1"), ("W2", "b2")):
                    w = cat_wb(inp[f"{s}_conv_{nm}"][l], inp[f"{s}_conv_{wf}"][l])
                    hi, lo = hilo(w)
                    m[f"{s}_conv_{nm}_{l}_hi"] = hi
                    m[f"{s}_conv_{nm}_{l}_lo"] = lo
        for d in ("mp", "pm"):
            W = np.asarray(inp[f"attn_{d}_W"], np.float32)
            bb = np.asarray(inp[f"attn_{d}_b"], np.float32)
            wq = cat_wb(W[0], bb[0]) * 0.25
            wv = cat_wb(W[2], bb[2])
            wk = cat_wb(W[1], bb[1])
            wka = np.zeros((65, 68), np.float32)
            for h in range(HEADS):
                wka[:, 17 * h:17 * h + 16] = wk[:, 16 * h:16 * h + 16]
                wka[64, 17 * h + 16] = 1.0
            for nm, w in (("q", wq), ("k", wka), ("v", wv)):
                hi, lo = hilo(w)
                m[f"attn_{d}_W{nm}_hi"] = hi
                m[f"attn_{d}_W{nm}_lo"] = lo
        percore.append(m)

    meta = dict(mol_T_blk=mol["T_blk"], mol_T_total=mol["T_total"],
                mol_E_core=mol["E_core"], mol_Wea=mol["Wea"],
                prot_T_blk=prot["T_blk"], prot_T_total=prot["T_total"],
                prot_E_core=prot["E_core"], prot_Wea=prot["Wea"])
    return meta, percore


# ------------------------------------------------------------- device build

def _build(meta):
    import concourse.bacc as bacc
    import concourse.mybir as mybir
    import concourse.tile as tile
    from concourse.bass import ds

    F32 = mybir.dt.float32
    BF16 = mybir.dt.bfloat16
    I32 = mybir.dt.int32
    AF = mybir.ActivationFunctionType
    ALU = mybir.AluOpType
    AX = mybir.AxisListType

    nc = bacc.Bacc("TRN2", target_bir_lowering=False, debug=False,
                   num_devices=R)

    dram = {}

    def din(name, shape, dtype=F32):
        dram[name] = nc.dram_tensor(name, list(shape), dtype,
                                    kind="ExternalInput")
        return dram[name]

    mT, mE = meta["mol_T_total"], meta["mol_E_core"]
    pT, pE = meta["prot_T_total"], meta["prot_E_core"]

    din("mol_xT", [12, NC_MOL]); din("prot_xT", [16, NC_PROT])
    din("mol_eaT", [75, meta["mol_Wea"]], BF16)
    din("prot_eaT", [75, meta["prot_Wea"]], BF16)
    din("mol_ohmod", [128, mE], BF16); din("prot_ohmod", [128, pE], BF16)
    din("mol_ohdst", [128, mT, 128], BF16)
    din("prot_ohdst", [128, pT, 128], BF16)
    din("mol_segblk", [1, 2 * mT], I32); din("prot_segblk", [1, 2 * pT], I32)
    din("mol_pmat", [128, NBLK_MOL, B])
    din("prot_pmat", [128, NBLK_PROT, B])
    din("ident_bf", [128, 128], BF16); din("ident_f32", [128, 128])
    din("node_lin_mol_W", [12, 64]); din("node_lin_prot_W", [16, 64])
    din("edge_lin_mol_W", [75, 64], BF16)
    din("edge_lin_prot_W", [75, 64], BF16)
    for s in ("mol", "prot"):
        for l in range(3):
            for nm in ("W1", "W2"):
                din(f"{s}_conv_{nm}_{l}_hi", [65, 64], BF16)
                din(f"{s}_conv_{nm}_{l}_lo", [65, 64], BF16)
    for d in ("mp", "pm"):
        for p in ("hi", "lo"):
            din(f"attn_{d}_Wq_{p}", [65, 64], BF16)
            din(f"attn_{d}_Wk_{p}", [65, 68], BF16)
            din(f"attn_{d}_Wv_{p}", [65, 64], BF16)
    din("fc1_W", [128, 64]); din("fc1_b", [64])
    din("fc2_W", [64, 1]); din("fc2_b", [1])

    out_d = nc.dram_tensor("out", [1, B], F32, kind="ExternalOutput")

    sides = {
        "mol": dict(N=N_MOL, NC=NC_MOL, nblk=NBLK_MOL, nbf=NBF_MOL,
                    T_blk=meta["mol_T_blk"], T_total=mT, E_core=mE, din=12),
        "prot": dict(N=N_PROT, NC=NC_PROT, nblk=NBLK_PROT, nbf=NBF_PROT,
                     T_blk=meta["prot_T_blk"], T_total=pT, E_core=pE, din=16),
    }

    with tile.TileContext(nc) as tc:
        const = tc.alloc_tile_pool(name="const", bufs=1)

        def load_const(name, shape, dtype=F32):
            t = const.tile(list(shape), dtype, name=f"c_{name}")
            nc.sync.dma_start(t[:], dram[name][:])
            return t

        ident_bf = load_const("ident_bf", [128, 128], BF16)
        ident_f32 = load_const("ident_f32", [128, 128])

        Wn = {"mol": load_const("node_lin_mol_W", [12, 64]),
              "prot": load_const("node_lin_prot_W", [16, 64])}
        W1 = {s: [[load_const(f"{s}_conv_W1_{l}_{p}", [65, 64], BF16)
                   for p in ("hi", "lo")] for l in range(3)] for s in sides}
        W2 = {s: [[load_const(f"{s}_conv_W2_{l}_{p}", [65, 64], BF16)
                   for p in ("hi", "lo")] for l in range(3)] for s in sides}
        sb_pmat = {s: load_const(f"{s}_pmat", [128, sides[s]["nblk"], B])
                   for s in sides}

        # ---------------- DRAM internals (bf16 node-major x)
        dpool = tc.alloc_tile_pool(name="dram", bufs=1, space="DRAM")
        x_sh_d = {s: [dpool.tile([sides[s]["NC"], 64 if l < 3 else 128], BF16,
                                 name=f"xsh_{s}_{l}") for l in range(4)]
                  for s in sides}
        x_full_d = {s: [dpool.tile([sides[s]["N"], 64 if l < 3 else 128], BF16,
                                   addr_space="Shared", name=f"xfull_{s}_{l}")
                        for l in range(4)] for s in sides}

        # ---------------- SBUF pools
        xT_pool = tc.alloc_tile_pool(name="xT", bufs=2)
        xnf_pool = tc.alloc_tile_pool(name="xnf", bufs=2)
        xfull_pool = tc.alloc_tile_pool(name="xfull", bufs=1)
        gmem = tc.alloc_tile_pool(name="gmem", bufs=1)
        msg_pool = tc.alloc_tile_pool(name="msg", bufs=3)
        gconst = tc.alloc_tile_pool(name="gconst", bufs=1)

        def load_gconst(name, shape, dtype=F32):
            t = gconst.tile(list(shape), dtype, name=f"g_{name}")
            nc.sync.dma_start(t[:], dram[name][:])
            return t

        We = {"mol": load_gconst("edge_lin_mol_W", [75, 64], BF16),
              "prot": load_gconst("edge_lin_prot_W", [75, 64], BF16)}
        sb_xTin = {"mol": load_gconst("mol_xT", [12, NC_MOL]),
                   "prot": load_gconst("prot_xT", [16, NC_PROT])}
        sb_seg = {s: load_gconst(f"{s}_segblk", [1, 2 * sides[s]["T_total"]],
                                 I32) for s in sides}
        sb_eaT, sb_ohmod, sb_ohdst = {}, {}, {}
        for s in sides:
            sd = sides[s]
            sb_eaT[s] = load_gconst(f"{s}_eaT", [75, meta[f"{s}_Wea"]], BF16)
            sb_ohmod[s] = load_gconst(f"{s}_ohmod", [128, sd["E_core"]], BF16)
            sb_ohdst[s] = load_gconst(f"{s}_ohdst",
                                      [128, sd["T_total"], 128], BF16)

        msgps = tc.alloc_tile_pool(name="msgps", bufs=2, space="PSUM")
        aggps = tc.alloc_tile_pool(name="aggps", bufs=2, space="PSUM")
        mlpps = tc.alloc_tile_pool(name="mlpps", bufs=2, space="PSUM")
        trps = tc.alloc_tile_pool(name="trps", bufs=2, space="PSUM")

        def xT_to_xnf_and_gather(s, l, xThi, xTlo=None):
            """Transpose xT shard to node-major, publish shard, AllGather.
            Last layer publishes hi|lo side by side in 128 feature cols."""
            sd = sides[s]
            nblk = sd["nblk"]
            fw = 64 if xTlo is None else 128
            xnf = xnf_pool.tile([128, nblk, fw], BF16, name=f"xnf_{s}_{fw}",
                                tag=f"xnf_{s}")
            for b in range(nblk):
                tp = trps.tile([128, 64], BF16, name="tr_ps")
                nc.tensor.transpose(tp[:], xThi[0:64, b * 128:(b + 1) * 128],
                                    ident_bf[0:64, 0:64])
                nc.vector.tensor_copy(xnf[:, b, 0:64], tp[:])
                if xTlo is not None:
                    tp2 = trps.tile([128, 64], BF16, name="tr_ps")
                    nc.tensor.transpose(tp2[:],
                                        xTlo[0:64, b * 128:(b + 1) * 128],
                                        ident_bf[0:64, 0:64])
                    nc.vector.tensor_copy(xnf[:, b, 64:128], tp2[:])
            nc.sync.dma_start(
                x_sh_d[s][l][:].rearrange("(t p) f -> p t f", p=128), xnf[:])
            nc.gpsimd.collective_compute(
                "AllGather", ALU.bypass, replica_groups=[list(range(R))],
                ins=[x_sh_d[s][l][:].opt()], outs=[x_full_d[s][l][:].opt()])
            return xnf

        def load_xfull(s, l):
            sd = sides[s]
            fw = 64 if l < 3 else 128
            xf = xfull_pool.tile([128, sd["nbf"], fw], BF16,
                                 name=f"xf_{s}_{fw}", tag=f"xf_{s}")
            nc.sync.dma_start(
                xf[:], x_full_d[s][l][:].rearrange("(t p) f -> p t f", p=128))
            return xf

        # initial node features x0 (no relu); x kept as f32 + bf16 hi/lo
        def make_triple(s, ps_ap, NCs, act):
            xTf = xT_pool.tile([65, NCs], F32, name=f"xTf_{s}", tag=f"xTf_{s}")
            nc.scalar.activation(xTf[0:64, :], ps_ap, act)
            xThi = xT_pool.tile([65, NCs], BF16, name=f"xTh_{s}",
                                tag=f"xTh_{s}")
            nc.scalar.activation(xThi[0:64, :], xTf[0:64, :], AF.Copy)
            xTlo = xT_pool.tile([65, NCs], BF16, name=f"xTl_{s}",
                                tag=f"xTl_{s}")
            nc.vector.tensor_sub(xTlo[0:64, :], xTf[0:64, :], xThi[0:64, :])
            nc.vector.memset(xThi[64:65, :], 1.0)
            nc.vector.memset(xTlo[64:65, :], 0.0)
            return xTf, xThi, xTlo

        xT_cur = {}
        xnf_cur = {}
        for s in sides:
            sd = sides[s]
            NCs = sd["NC"]
            ps = mlpps.tile([64, 512], F32, name="mlp_ps")
            nc.tensor.matmul(ps[:, 0:NCs], Wn[s][:], sb_xTin[s][:],
                             start=True, stop=True)
            xT_cur[s] = make_triple(s, ps[:, 0:NCs], NCs, AF.Copy)
            xnf_cur[s] = xT_to_xnf_and_gather(s, 0, xT_cur[s][1])

        # ---------------- GINE layers
        segregs = [nc.tensor.alloc_register(f"segreg{i}") for i in range(16)]

        def seg_vals16(ap, n, hi):
            regs = segregs[:n]
            nc.tensor.reg_load(regs, ap)
            return [nc.tensor.snap(r, donate=True, min_val=0, max_val=hi)
                    for r in regs]

        for l in range(3):
            for s in ("prot", "mol"):
                sd = sides[s]
                NCs, nblk, T_blk, nbf = sd["NC"], sd["nblk"], sd["T_blk"], sd["nbf"]
                xfull = load_xfull(s, l)
                xTf_prev = xT_cur[s][0]
                hTf = gmem.tile([65, NCs], F32, name=f"hTf_{s}",
                                tag=f"hTf_{s}")
                for b in range(nblk):
                    agg = aggps.tile([64, 128], F32, name="agg_ps")
                    for g0 in range(0, T_blk, 8):
                        ng = min(8, T_blk - g0)
                        t0 = b * T_blk + g0
                        vals = seg_vals16(
                            sb_seg[s][0:1, 2 * t0:2 * t0 + 2 * ng],
                            2 * ng, nbf - 1)
                        mps = msgps.tile([128, 8, 64], F32, name="msg_ps")
                        for j in range(ng):
                            t = t0 + j
                            e0 = t * 128
                            nc.tensor.matmul(
                                mps[0:64, j, :],
                                sb_ohmod[s][:, e0:e0 + 64],
                                xfull[:, ds(vals[2 * j], 1), :],
                                start=True, stop=False, tile_position=(0, 0),
                                skip_group_check=True)
                            nc.tensor.matmul(
                                mps[64:128, j, :],
                                sb_ohmod[s][:, e0 + 64:e0 + 128],
                                xfull[:, ds(vals[2 * j + 1], 1), :],
                                start=True, stop=False, tile_position=(0, 64),
                                skip_group_check=True)
                            rb, cb = 64 * (t % 2), 128 * (t // 2)
                            nc.tensor.matmul(
                                mps[:, j, :],
                                sb_eaT[s][rb:rb + 11, cb:cb + 128],
                                We[s][rb:rb + 11, :], start=False, stop=True,
                                skip_group_check=True)
                        msg = msg_pool.tile([128, 8, 64], BF16, name="msg_sb")
                        nc.scalar.activation(msg[:, 0:ng, :], mps[:, 0:ng, :],
                                             AF.Relu)
                        for j in range(ng):
                            t = b * T_blk + g0 + j
                            nc.tensor.matmul(
                                agg[:], msg[:, j, :], sb_ohdst[s][:, t, :],
                                start=(g0 + j == 0),
                                stop=(g0 + j == T_blk - 1),
                                skip_group_check=True)
                    nc.vector.tensor_add(hTf[0:64, b * 128:(b + 1) * 128],
                                         xTf_prev[0:64, b * 128:(b + 1) * 128],
                                         agg[:])
                hThi = gmem.tile([65, NCs], BF16, name=f"hTh_{s}",
                                 tag=f"hTh_{s}")
                nc.scalar.activation(hThi[0:64, :], hTf[0:64, :], AF.Copy)
                hTlo = gmem.tile([65, NCs], BF16, name=f"hTl_{s}",
                                 tag=f"hTl_{s}")
                nc.vector.tensor_sub(hTlo[0:64, :], hTf[0:64, :],
                                     hThi[0:64, :])
                nc.vector.memset(hThi[64:65, :], 1.0)
                nc.vector.memset(hTlo[64:65, :], 0.0)

                def mlp3(Wp, rhs_hi, rhs_lo, NCs):
                    ps_ = mlpps.tile([64, 512], F32, name="mlp_ps")
                    nc.tensor.matmul(ps_[:, 0:NCs], Wp[0][:], rhs_hi[:],
                                     start=True, stop=False,
                                     skip_group_check=True)
                    nc.tensor.matmul(ps_[:, 0:NCs], Wp[0][:], rhs_lo[:],
                                     start=False, stop=False,
                                     skip_group_check=True)
                    nc.tensor.matmul(ps_[:, 0:NCs], Wp[1][:], rhs_hi[:],
                                     start=False, stop=True,
                                     skip_group_check=True)
                    return ps_

                ps1 = mlp3(W1[s][l], hThi, hTlo, NCs)
                r1f = gmem.tile([65, NCs], F32, name=f"r1f_{s}",
                                tag=f"r1f_{s}")
                nc.scalar.activation(r1f[0:64, :], ps1[:, 0:NCs], AF.Relu)
                r1hi = gmem.tile([65, NCs], BF16, name=f"r1h_{s}",
                                 tag=f"r1h_{s}")
                nc.scalar.activation(r1hi[0:64, :], r1f[0:64, :], AF.Copy)
                r1lo = gmem.tile([65, NCs], BF16, name=f"r1l_{s}",
                                 tag=f"r1l_{s}")
                nc.vector.tensor_sub(r1lo[0:64, :], r1f[0:64, :],
                                     r1hi[0:64, :])
                nc.vector.memset(r1hi[64:65, :], 1.0)
                nc.vector.memset(r1lo[64:65, :], 0.0)
                ps2 = mlp3(W2[s][l], r1hi, r1lo, NCs)
                xT_cur[s] = make_triple(s, ps2[:, 0:NCs], NCs, AF.Relu)
                xnf_cur[s] = xT_to_xnf_and_gather(
                    s, l + 1, xT_cur[s][1],
                    xT_cur[s][2] if l == 2 else None)

        for p in (trps, mlpps, aggps, msgps):
            p.release()
        gconst.release()
        msg_pool.release()
        gmem.release()

        # ---------------- attention phase
        a_sb = tc.alloc_tile_pool(name="attn_sb", bufs=1)
        smallps = tc.alloc_tile_pool(name="smallps", bufs=2, space="PSUM")
        s12ps = tc.alloc_tile_pool(name="s12ps", bufs=2, space="PSUM")
        ops = tc.alloc_tile_pool(name="ops", bufs=1, space="PSUM")
        ex_pool = tc.alloc_tile_pool(name="expt", bufs=6)

        def sps():
            return smallps.tile([128, 512], F32, name="small_ps")

        def sbf():
            return smallps.tile([128, 128], BF16, name="small_bf")

        # final x of both sides: load node-major hi|lo, build transposed pair
        xT_full = {}
        for s in sides:
            sd = sides[s]
            Ns, nbf = sd["N"], sd["nbf"]
            xf = load_xfull(s, 3)
            xT_fh = a_sb.tile([65, Ns], BF16, name=f"xTfullh_{s}")
            xT_fl = a_sb.tile([65, Ns], BF16, name=f"xTfulll_{s}")
            for t in range(nbf):
                tp = sbf()
                nc.tensor.transpose(tp[0:64, 0:128], xf[:, t, 0:64],
                                    ident_bf[:])
                nc.vector.tensor_copy(xT_fh[0:64, t * 128:(t + 1) * 128],
                                      tp[0:64, 0:128])
                tp2 = sbf()
                nc.tensor.transpose(tp2[0:64, 0:128], xf[:, t, 64:128],
                                    ident_bf[:])
                nc.vector.tensor_copy(xT_fl[0:64, t * 128:(t + 1) * 128],
                                      tp2[0:64, 0:128])
            nc.vector.memset(xT_fh[64:65, :], 1.0)
            nc.vector.memset(xT_fl[64:65, :], 0.0)
            xT_full[s] = (xT_fh, xT_fl)

        H_sb = {}
        for dirn, (qs, ks) in (("mp", ("mol", "prot")), ("pm", ("prot", "mol"))):
            qd, kd = sides[qs], sides[ks]
            NCq, Nk = qd["NC"], kd["N"]
            n_qt = NCq // 128
            n_k128 = Nk // 128
            n_k512 = Nk // 512
            Wq, Wk, Wv = [], [], []
            for p, lst, wd in (("hi", Wq, 64), ("lo", Wq, 64),
                               ("hi", Wk, 68), ("lo", Wk, 68),
                               ("hi", Wv, 64), ("lo", Wv, 64)):
                nm = "q" if lst is Wq else ("k" if lst is Wk else "v")
                t = a_sb.tile([65, wd], BF16, name=f"W{nm}{p}_{dirn}")
                nc.sync.dma_start(t[:], dram[f"attn_{dirn}_W{nm}_{p}"][:])
                lst.append(t)

            def mm3w(out_ap, Wpair, csl, xpair, rsl, NCo):
                nc.tensor.matmul(out_ap, Wpair[0][:, csl], xpair[0][:, rsl],
                                 start=True, stop=False,
                                 skip_group_check=True)
                nc.tensor.matmul(out_ap, Wpair[0][:, csl], xpair[1][:, rsl],
                                 start=False, stop=False,
                                 skip_group_check=True)
                nc.tensor.matmul(out_ap, Wpair[1][:, csl], xpair[0][:, rsl],
                                 start=False, stop=True,
                                 skip_group_check=True)

            # K^T per head with ones row (via augmented Wk), hi/lo pair
            KTh = [a_sb.tile([81, Nk], BF16, name=f"KTh_{dirn}_{i}")
                   for i in range(2)]
            KTl = [a_sb.tile([81, Nk], BF16, name=f"KTl_{dirn}_{i}")
                   for i in range(2)]
            for h in range(HEADS):
                r0 = 64 * (h % 2)
                for cc in range(n_k512):
                    csl = slice(cc * 512, (cc + 1) * 512)
                    pk = sps()[r0:r0 + 17, :]
                    mm3w(pk[:], Wk, slice(17 * h, 17 * h + 17),
                         xT_full[ks], csl, 512)
                    nc.scalar.activation(KTh[h // 2][r0:r0 + 17, csl],
                                         pk[:], AF.Copy)
                    nc.vector.tensor_sub(KTl[h // 2][r0:r0 + 17, csl],
                                         pk[:], KTh[h // 2][r0:r0 + 17, csl])

            # Q^T per head (0.25 folded in Wq); hi has the -m row, plus lo
            QTo = [a_sb.tile([81, NCq], BF16, name=f"QTo_{dirn}_{i}")
                   for i in range(2)]
            QTl = [a_sb.tile([81, NCq], BF16, name=f"QTl_{dirn}_{i}")
                   for i in range(2)]
            for h in range(HEADS):
                r0 = 64 * (h % 2)
                pq = sps()[r0:r0 + 16, :]
                mm3w(pq[:, 0:NCq], Wq, slice(16 * h, 16 * h + 16),
                     (xT_cur[qs][1], xT_cur[qs][2]), slice(0, NCq), NCq)
                nc.scalar.activation(QTo[h // 2][r0:r0 + 16, :], pq[:, 0:NCq],
                                     AF.Copy)
                nc.vector.tensor_sub(QTl[h // 2][r0:r0 + 16, :], pq[:, 0:NCq],
                                     QTo[h // 2][r0:r0 + 16, :])
            for h in range(HEADS):
                QTt, KTt, r0 = QTo[h // 2], KTh[h // 2], 64 * (h % 2)
                negm = a_sb.tile([1, NCq], BF16, name="negm", bufs=2,
                                 tag="negm")
                for qt in range(n_qt):
                    mx = a_sb.tile([128, n_k512], F32, name="mx", bufs=2,
                                   tag="mx")
                    for cch in range(n_k512):
                        spT = s12ps.tile([128, 512], F32, name="s_ps")
                        nc.tensor.matmul(
                            spT[:],
                            QTt[r0:r0 + 16, qt * 128:(qt + 1) * 128],
                            KTt[r0:r0 + 16, cch * 512:(cch + 1) * 512],
                            start=True, stop=True)
                        nc.vector.reduce_max(mx[:, cch:cch + 1], spT[:],
                                             axis=AX.X)
                    mqt = a_sb.tile([128, 1], F32, name="mqt", bufs=2,
                                    tag="mqt")
                    nc.vector.reduce_max(mqt[:], mx[:], axis=AX.X)
                    tpm = sps()[0:1, 0:128]
                    nc.tensor.transpose(tpm[:], mqt[:], ident_f32[:])
                    nc.scalar.activation(negm[0:1, qt * 128:(qt + 1) * 128],
                                         tpm[:], AF.Copy, scale=-1.0)
                nc.sync.dma_start(QTt[r0 + 16:r0 + 17, :], negm[:])

            # V' [128, n_k128, 4, 34]: cols 0-15 Vhi, 16 ones, 17-32 Vlo,
            # 33 zero; hi and lo halves feed two accumulating wV matmuls
            Vp = a_sb.tile([128, n_k128, HEADS, 34], BF16, name=f"Vp_{dirn}")
            nc.vector.memset(Vp[:, :, :, 16:17], 1.0)
            nc.vector.memset(Vp[:, :, :, 33:34], 0.0)
            for kt in range(n_k128):
                ksl = slice(kt * 128, (kt + 1) * 128)
                pv = sps()[:, 0:64]
                nc.tensor.matmul(pv[:], xT_full[ks][0][:, ksl], Wv[0][:],
                                 start=True, stop=False,
                                 skip_group_check=True)
                nc.tensor.matmul(pv[:], xT_full[ks][1][:, ksl], Wv[0][:],
                                 start=False, stop=False,
                                 skip_group_check=True)
                nc.tensor.matmul(pv[:], xT_full[ks][0][:, ksl], Wv[1][:],
                                 start=False, stop=True,
                                 skip_group_check=True)
                nc.scalar.activation(
                    Vp[:, kt, :, 0:16],
                    pv[:].rearrange("p (h d) -> p h d", h=HEADS), AF.Copy)
                nc.vector.tensor_sub(
                    Vp[:, kt, :, 17:33],
                    pv[:].rearrange("p (h d) -> p h d", h=HEADS),
                    Vp[:, kt, :, 0:16])

            # scores (hi.hi + hi.lo + lo.hi) -> exp -> wV, single pass
            o_ps = [ops.tile([81, NCq], F32, name=f"o_ps_{i}")
                    for i in range(2)]
            for kc in range(n_k128):
                ksl = slice(kc * 128, (kc + 1) * 128)
                for h in range(HEADS):
                    i2, r0 = h // 2, 64 * (h % 2)
                    sp = s12ps.tile([128, 512], F32, name="s_ps")[:, 0:NCq]
                    nc.tensor.matmul(sp[:], KTh[i2][r0:r0 + 17, ksl],
                                     QTo[i2][r0:r0 + 17, :],
                                     start=True, stop=False,
                                     skip_group_check=True)
                    nc.tensor.matmul(sp[:], KTh[i2][r0:r0 + 16, ksl],
                                     QTl[i2][r0:r0 + 16, :],
                                     start=False, stop=False,
                                     skip_group_check=True)
                    nc.tensor.matmul(sp[:], KTl[i2][r0:r0 + 16, ksl],
                                     QTo[i2][r0:r0 + 16, :],
                                     start=False, stop=True,
                                     skip_group_check=True)
                    ex = ex_pool.tile([128, NCq], BF16, name="ex",
                                      tag=f"ex_{dirn}")
                    nc.scalar.activation(ex[:], sp[:], AF.Exp)
                    nc.tensor.matmul(o_ps[i2][r0:r0 + 17, :],
                                     Vp[:, kc, h, 0:17], ex[:],
                                     start=(kc == 0), stop=False,
                                     skip_group_check=True)
                    nc.tensor.matmul(o_ps[i2][r0:r0 + 17, :],
                                     Vp[:, kc, h, 17:34], ex[:],
                                     start=False, stop=(kc == n_k128 - 1),
                                     skip_group_check=True)

            # normalize + assemble H (node-major, f32) + residual hi+lo
            H = a_sb.tile([128, n_qt, 64], F32, name=f"H_{dirn}")
            for h in range(HEADS):
                ro = 64 * (h % 2)
                osb = a_sb.tile([81, NCq], F32, name="osb", bufs=2, tag="osb")
                nc.vector.tensor_copy(osb[ro:ro + 17, :],
                                      o_ps[h // 2][ro:ro + 17, :])
                for qt in range(n_qt):
                    tp = sps()[:, 0:17]
                    nc.tensor.transpose(tp[:],
                                        osb[ro:ro + 17,
                                            qt * 128:(qt + 1) * 128],
                                        ident_f32[ro:ro + 17, ro:ro + 17])
                    inv1 = a_sb.tile([128, 1], F32, name="inv1", bufs=2,
                                     tag="inv1")
                    nc.vector.reciprocal(inv1[:], tp[:, 16:17])
                    nc.vector.tensor_scalar_mul(
                        H[:, qt, 16 * h:16 * (h + 1)], tp[:, 0:16], inv1[:])
            nc.vector.tensor_add(H[:], H[:], xnf_cur[qs][:, :, 0:64])
            nc.vector.tensor_add(H[:], H[:], xnf_cur[qs][:, :, 64:128])
            H_sb[dirn] = H

        # ---------------- pooling + output MLP
        zt_part_d = dpool.tile([128, B], F32, name="zt_part")
        zt_full_d = dpool.tile([128, B], F32, addr_space="Shared",
                               name="zt_full")
        for dirn, qs in (("mp", "mol"), ("pm", "prot")):
            n_qt = sides[qs]["NC"] // 128
            psz = sps()[0:64, 0:B]
            for qt in range(n_qt):
                nc.tensor.matmul(psz[:], H_sb[dirn][:, qt, :],
                                 sb_pmat[qs][:, qt, :],
                                 start=(qt == 0), stop=(qt == n_qt - 1),
                                 skip_group_check=True)
            zpart = a_sb.tile([64, B], F32, name=f"zpart_{dirn}")
            nc.vector.tensor_copy(zpart[:], psz[:])
            row0 = 0 if dirn == "mp" else 64
            nc.sync.dma_start(zt_part_d[row0:row0 + 64, :], zpart[:])
        nc.gpsimd.collective_compute(
            "AllReduce", ALU.add, replica_groups=[list(range(R))],
            ins=[zt_part_d[:].opt()], outs=[zt_full_d[:].opt()])
        zT = a_sb.tile([128, B], F32, name="zT")
        nc.sync.dma_start(zT[:], zt_full_d[:])

        fc1W = a_sb.tile([128, 64], F32, name="fc1W")
        nc.sync.dma_start(fc1W[:], dram["fc1_W"][:])
        fc1b = a_sb.tile([64, 1], F32, name="fc1b")
        nc.sync.dma_start(fc1b[:], dram["fc1_b"][:, None])
        fc2W = a_sb.tile([64, 1], F32, name="fc2W")
        nc.sync.dma_start(fc2W[:], dram["fc2_W"][:])
        fc2b = a_sb.tile([1, 1], F32, name="fc2b")
        nc.sync.dma_start(fc2b[:], dram["fc2_b"][:, None])

        ps = sps()[0:64, 0:B]
        nc.tensor.matmul(ps[:], fc1W[:], zT[:], start=True, stop=True)
        h1 = a_sb.tile([65, B], F32, name="h1")
        nc.scalar.activation(h1[0:64, :], ps[:], AF.Relu, bias=fc1b[:])
        ps2 = sps()[0:1, 0:B]
        nc.tensor.matmul(ps2[:], fc2W[:], h1[0:64, :], start=True, stop=True)
        osb = a_sb.tile([1, B], F32, name="osb_out")
        nc.scalar.activation(osb[:], ps2[:], AF.Sigmoid, bias=fc2b[:])
        nc.sync.dma_start(out_d[:], osb[:])

        ex_pool.release()
        ops.release()
        s12ps.release()
        smallps.release()
        a_sb.release()
        xfull_pool.release()
        xnf_pool.release()
        xT_pool.release()
        dpool.release()
        const.release()

    nc.compile()
    return nc


# ----------------------------------------------------------------- entry

def kernel(**inputs):
    global last_results
    meta, percore = _prep_host(inputs)
    key = (meta["mol_T_blk"], meta["prot_T_blk"])
    if key not in _CACHE:
        _CACHE[key] = _build(meta)
    nc = _CACHE[key]
    from concourse.bass_utils import run_bass_kernel_spmd
    res = run_bass_kernel_spmd(nc, percore, list(range(R)))
    last_results = res
    return np.asarray(res.results[0]["out"], np.float32).reshape(B)


# revision 22
# speedup vs baseline: 1.3191x; 1.0602x over previous
"""CrossGraphAttentionModel on 8 Trainium2 NeuronCores (Bass/Tile, SPMD).

Sharding: nodes/edges of both graphs are sharded 8 ways by (dst-sorted) node
range; 64-dim weights replicated, all in bf16 on the PE. Per GINE layer the
x[src] gather is done ON the PE: edges are sorted by (dst window, src block)
with runs padded to 64, so every 64-edge half-tile reads one 128-node block
of the AllGathered x; a host-built src-mod-128 one-hot (lhsT) contracts
against that block, whose index is loaded per half-tile from a per-core table
into PE registers (dynamic rhs offset). The edge-linear term accumulates into
the same PSUM via a second matmul, ACT applies the relu, and a dst one-hot
matmul scatter-adds messages per 128-node window. Cross-graph attention is
single-pass: scores use contraction-17 matmuls (K^T tiles with a ones row
against per-head Q with a folded Cauchy-Schwarz row bound), exp runs on ACT
into bf16, and the wV product accumulates transposed [17, Nq] outputs with a
ones column producing softmax denominators for free. Graph pooling is a
one-hot matmul with 1/count weights, AllReduced, then the tiny output MLP.
"""

import numpy as np
import ml_dtypes

BF = ml_dtypes.bfloat16

R = 8
HID = 64
B = 32
HEADS = 4
HD = 16
N_MOL, N_PROT = 2048, 4096
E_MOL, E_PROT = 32768, 131072
NC_MOL, NC_PROT = N_MOL // R, N_PROT // R          # 256, 512
NBLK_MOL, NBLK_PROT = NC_MOL // 128, NC_PROT // 128  # 2, 4
NBF_MOL, NBF_PROT = N_MOL // 128, N_PROT // 128      # 16, 32

_CACHE = {}
last_results = None


# ----------------------------------------------------------------- host prep

def _prep_edges(edge_index, eattr, nblk):
    """Sort edges by dst window, then by src block inside each window, with
    every (window, srcblk) run padded to a multiple of 64 edges. Produces the
    per-core one-hot operands and the per-half-tile source-block table."""
    src, dst = np.asarray(edge_index[0]), np.asarray(edge_index[1])
    eattr = np.asarray(eattr, np.float32)
    D = eattr.shape[1]
    nwin = R * nblk

    order = np.argsort(dst, kind="stable")
    src_s, dst_s, ea_s = src[order], dst[order], eattr[order]
    win = dst_s // 128
    counts = np.bincount(win, minlength=nwin)
    starts = np.concatenate([[0], np.cumsum(counts)])

    # per-window edge lists grouped by srcblk, runs padded to 64
    win_srcmod = []
    win_dstloc = []
    win_ea = []
    win_blk64 = []   # srcblk per 64-edge slot
    padded_len = np.zeros(nwin, np.int64)
    for g in range(nwin):
        lo, hi = starts[g], starts[g + 1]
        s_g, d_g, e_g = src_s[lo:hi], dst_s[lo:hi], ea_s[lo:hi]
        o2 = np.argsort(s_g // 128, kind="stable")
        s_g, d_g, e_g = s_g[o2], d_g[o2], e_g[o2]
        blk = s_g // 128
        sm_l, dl_l, ea_l, b64_l = [], [], [], []
        for bk in np.unique(blk):
            m = blk == bk
            n = int(m.sum())
            npad = -n % 64
            sm = np.concatenate([s_g[m] % 128, np.full(npad, -1, np.int64)])
            dl = np.concatenate([d_g[m] - g * 128, np.full(npad, -1, np.int64)])
            ea = np.concatenate([e_g[m], np.zeros((npad, D), np.float32)])
            sm_l.append(sm); dl_l.append(dl); ea_l.append(ea)
            b64_l.extend([int(bk)] * ((n + npad) // 64))
        win_srcmod.append(np.concatenate(sm_l) if sm_l else np.zeros(0, np.int64))
        win_dstloc.append(np.concatenate(dl_l) if dl_l else np.zeros(0, np.int64))
        win_ea.append(np.concatenate(ea_l) if ea_l else np.zeros((0, D), np.float32))
        win_blk64.append(b64_l)
        padded_len[g] = len(win_srcmod[-1])

    T_blk = max(1, int(np.ceil(padded_len.max() / 128)))
    T_total = nblk * T_blk
    E_core = T_total * 128

    Wea = ((T_total + 1) // 2) * 128
    ohmod = np.zeros((R, 128, E_core), BF)
    ohdst = np.zeros((R, 128, T_total, 128), BF)
    eaT = np.zeros((R, D + 1, E_core), np.float32)
    segblk = np.zeros((R, 1, 2 * T_total), np.int32)
    for c in range(R):
        for b in range(nblk):
            g = c * nblk + b
            sm, dl, ea = win_srcmod[g], win_dstloc[g], win_ea[g]
            n = len(sm)
            off = b * T_blk * 128
            e_idx = off + np.arange(n)
            real = sm >= 0
            ohmod[c, sm[real], e_idx[real]] = 1.0
            t_idx = e_idx // 128
            p_idx = e_idx % 128
            ohdst[c, p_idx[real], t_idx[real], dl[real]] = 1.0
            eaT[c, :D, off:off + n] = ea.T
            eaT[c, D, off:off + n] = real.astype(np.float32)
            b64 = win_blk64[g]
            h0 = (b * T_blk) * 2
            segblk[c, 0, h0:h0 + len(b64)] = b64
    # pack eaT 2-up: tile t -> rows 64*(t%2)..+11, cols (t//2)*128..+128
    eaT_w = np.zeros((R, 75, Wea), np.float32)
    for t in range(T_total):
        rb, cb = 64 * (t % 2), 128 * (t // 2)
        eaT_w[:, rb:rb + D + 1, cb:cb + 128] = \
            eaT[:, :, t * 128:(t + 1) * 128]
    return dict(T_blk=T_blk, T_total=T_total, E_core=E_core, D=D, Wea=Wea,
                ohmod=ohmod, ohdst=ohdst,
                eaT_w=np.ascontiguousarray(eaT_w.astype(BF)), segblk=segblk)


def _prep_host(inp):
    """All integer/layout preprocessing. Returns (meta, per_core_inputs)."""
    mol = _prep_edges(inp["mol_edge_index"], inp["mol_eattr"], NBLK_MOL)
    prot = _prep_edges(inp["prot_edge_index"], inp["prot_eattr"], NBLK_PROT)

    # pool matrices with 1/count entries, wrapped [128, nblk, B]
    def pmat(batch, ncore, nblk):
        batch = np.asarray(batch)
        cnt = np.bincount(batch, minlength=B).astype(np.float32)
        inv = 1.0 / np.maximum(cnt, 1.0)
        m = np.zeros((R, ncore, B), np.float32)
        for c in range(R):
            sl = batch[c * ncore:(c + 1) * ncore]
            m[c, np.arange(ncore), sl] = inv[sl]
        return np.ascontiguousarray(
            m.reshape(R, nblk, 128, B).transpose(0, 2, 1, 3))

    mol_pmat = pmat(inp["mol_batch"], NC_MOL, NBLK_MOL)
    prot_pmat = pmat(inp["prot_batch"], NC_PROT, NBLK_PROT)

    # node features transposed per core with ones row (fp32)
    def xt(x, ncore):
        x = np.asarray(x, np.float32)
        d = x.shape[1]
        out = np.zeros((R, d + 1, ncore), np.float32)
        for c in range(R):
            out[c, :d] = x[c * ncore:(c + 1) * ncore].T
            out[c, d] = 1.0
        return out

    mol_xT = xt(inp["mol_x"], NC_MOL)        # [R, 12, 256]
    prot_xT = xt(inp["prot_x"], NC_PROT)     # [R, 16, 512]

    ident_bf = np.eye(128, dtype=BF)
    ident_f32 = np.eye(128, dtype=np.float32)

    def cat_wb(W, b):  # -> [din+1, dout] fp32
        W = np.asarray(W, np.float32)
        b = np.asarray(b, np.float32)
        return np.concatenate([W, b[None, :]], 0)

    percore = []
    for c in range(R):
        m = {
            "mol_xT": mol_xT[c], "prot_xT": prot_xT[c],
            "mol_eaT": mol["eaT_w"][c], "prot_eaT": prot["eaT_w"][c],
            "mol_ohmod": mol["ohmod"][c], "prot_ohmod": prot["ohmod"][c],
            "mol_ohdst": mol["ohdst"][c], "prot_ohdst": prot["ohdst"][c],
            "mol_segblk": mol["segblk"][c], "prot_segblk": prot["segblk"][c],
            "mol_pmat": mol_pmat[c], "prot_pmat": prot_pmat[c],
            "ident_bf": ident_bf, "ident_f32": ident_f32,
            "node_lin_mol_W": cat_wb(inp["node_lin_mol_W"], inp["node_lin_mol_b"]),
            "node_lin_prot_W": cat_wb(inp["node_lin_prot_W"], inp["node_lin_prot_b"]),
            "edge_lin_mol_W": np.ascontiguousarray(np.tile(
                np.pad(cat_wb(inp["edge_lin_mol_W"], inp["edge_lin_mol_b"]),
                       ((0, 53), (0, 0))), (2, 1))[:75]).astype(BF),
            "edge_lin_prot_W": np.ascontiguousarray(np.tile(
                np.pad(cat_wb(inp["edge_lin_prot_W"], inp["edge_lin_prot_b"]),
                       ((0, 53), (0, 0))), (2, 1))[:75]).astype(BF),
            "fc1_W": np.asarray(inp["fc1_W"], np.float32),
            "fc1_b": np.asarray(inp["fc1_b"], np.float32),
            "fc2_W": np.asarray(inp["fc2_W"], np.float32),
            "fc2_b": np.asarray(inp["fc2_b"], np.float32),
        }
        def hilo(w):
            hi = w.astype(BF)
            lo = (w - hi.astype(np.float32)).astype(BF)
            return hi, lo
        for s in ("mol", "prot"):
            for l in range(3):
                for nm, wf in (("W1", "b1"), ("W2", "b2")):
                    w = cat_wb(inp[f"{s}_conv_{nm}"][l], inp[f"{s}_conv_{wf}"][l])
                    hi, lo = hilo(w)
                    m[f"{s}_conv_{nm}_{l}_hi"] = hi
                    m[f"{s}_conv_{nm}_{l}_lo"] = lo
        for d in ("mp", "pm"):
            W = np.asarray(inp[f"attn_{d}_W"], np.float32)
            bb = np.asarray(inp[f"attn_{d}_b"], np.float32)
            wq = cat_wb(W[0], bb[0]) * 0.25
            wv = cat_wb(W[2], bb[2])
            wk = cat_wb(W[1], bb[1])
            wka = np.zeros((65, 68), np.float32)
            for h in range(HEADS):
                wka[:, 17 * h:17 * h + 16] = wk[:, 16 * h:16 * h + 16]
                wka[64, 17 * h + 16] = 1.0
            for nm, w in (("q", wq), ("k", wka), ("v", wv)):
                hi, lo = hilo(w)
                m[f"attn_{d}_W{nm}_hi"] = hi
                m[f"attn_{d}_W{nm}_lo"] = lo
        percore.append(m)

    meta = dict(mol_T_blk=mol["T_blk"], mol_T_total=mol["T_total"],
                mol_E_core=mol["E_core"], mol_Wea=mol["Wea"],
                prot_T_blk=prot["T_blk"], prot_T_total=prot["T_total"],
                prot_E_core=prot["E_core"], prot_Wea=prot["Wea"])
    return meta, percore


# ------------------------------------------------------------- device build

def _build(meta):
    import concourse.bacc as bacc
    import concourse.mybir as mybir
    import concourse.tile as tile
    from concourse.bass import ds

    F32 = mybir.dt.float32
    BF16 = mybir.dt.bfloat16
    I32 = mybir.dt.int32
    AF = mybir.ActivationFunctionType
    ALU = mybir.AluOpType
    AX = mybir.AxisListType

    nc = bacc.Bacc("TRN2", target_bir_lowering=False, debug=False,
                   num_devices=R)

    dram = {}

    def din(name, shape, dtype=F32):
        dram[name] = nc.dram_tensor(name, list(shape), dtype,
                                    kind="ExternalInput")
        return dram[name]

    mT, mE = meta["mol_T_total"], meta["mol_E_core"]
    pT, pE = meta["prot_T_total"], meta["prot_E_core"]

    din("mol_xT", [12, NC_MOL]); din("prot_xT", [16, NC_PROT])
    din("mol_eaT", [75, meta["mol_Wea"]], BF16)
    din("prot_eaT", [75, meta["prot_Wea"]], BF16)
    din("mol_ohmod", [128, mE], BF16); din("prot_ohmod", [128, pE], BF16)
    din("mol_ohdst", [128, mT, 128], BF16)
    din("prot_ohdst", [128, pT, 128], BF16)
    din("mol_segblk", [1, 2 * mT], I32); din("prot_segblk", [1, 2 * pT], I32)
    din("mol_pmat", [128, NBLK_MOL, B])
    din("prot_pmat", [128, NBLK_PROT, B])
    din("ident_bf", [128, 128], BF16); din("ident_f32", [128, 128])
    din("node_lin_mol_W", [12, 64]); din("node_lin_prot_W", [16, 64])
    din("edge_lin_mol_W", [75, 64], BF16)
    din("edge_lin_prot_W", [75, 64], BF16)
    for s in ("mol", "prot"):
        for l in range(3):
            for nm in ("W1", "W2"):
                din(f"{s}_conv_{nm}_{l}_hi", [65, 64], BF16)
                din(f"{s}_conv_{nm}_{l}_lo", [65, 64], BF16)
    for d in ("mp", "pm"):
        for p in ("hi", "lo"):
            din(f"attn_{d}_Wq_{p}", [65, 64], BF16)
            din(f"attn_{d}_Wk_{p}", [65, 68], BF16)
            din(f"attn_{d}_Wv_{p}", [65, 64], BF16)
    din("fc1_W", [128, 64]); din("fc1_b", [64])
    din("fc2_W", [64, 1]); din("fc2_b", [1])

    out_d = nc.dram_tensor("out", [1, B], F32, kind="ExternalOutput")

    sides = {
        "mol": dict(N=N_MOL, NC=NC_MOL, nblk=NBLK_MOL, nbf=NBF_MOL,
                    T_blk=meta["mol_T_blk"], T_total=mT, E_core=mE, din=12),
        "prot": dict(N=N_PROT, NC=NC_PROT, nblk=NBLK_PROT, nbf=NBF_PROT,
                     T_blk=meta["prot_T_blk"], T_total=pT, E_core=pE, din=16),
    }

    with tile.TileContext(nc) as tc:
        const = tc.alloc_tile_pool(name="const", bufs=1)

        def load_const(name, shape, dtype=F32):
            t = const.tile(list(shape), dtype, name=f"c_{name}")
            nc.sync.dma_start(t[:], dram[name][:])
            return t

        ident_bf = load_const("ident_bf", [128, 128], BF16)
        ident_f32 = load_const("ident_f32", [128, 128])

        Wn = {"mol": load_const("node_lin_mol_W", [12, 64]),
              "prot": load_const("node_lin_prot_W", [16, 64])}
        W1 = {s: [[load_const(f"{s}_conv_W1_{l}_{p}", [65, 64], BF16)
                   for p in ("hi", "lo")] for l in range(3)] for s in sides}
        W2 = {s: [[load_const(f"{s}_conv_W2_{l}_{p}", [65, 64], BF16)
                   for p in ("hi", "lo")] for l in range(3)] for s in sides}
        sb_pmat = {s: load_const(f"{s}_pmat", [128, sides[s]["nblk"], B])
                   for s in sides}

        # ---------------- DRAM internals (bf16 node-major x)
        dpool = tc.alloc_tile_pool(name="dram", bufs=1, space="DRAM")
        x_sh_d = {s: [dpool.tile([sides[s]["NC"], 64 if l < 3 else 128], BF16,
                                 name=f"xsh_{s}_{l}") for l in range(4)]
                  for s in sides}
        x_full_d = {s: [dpool.tile([sides[s]["N"], 64 if l < 3 else 128], BF16,
                                   addr_space="Shared", name=f"xfull_{s}_{l}")
                        for l in range(4)] for s in sides}

        # ---------------- SBUF pools
        xT_pool = tc.alloc_tile_pool(name="xT", bufs=2)
        xnf_pool = tc.alloc_tile_pool(name="xnf", bufs=2)
        xfull_pool = tc.alloc_tile_pool(name="xfull", bufs=1)
        gmem = tc.alloc_tile_pool(name="gmem", bufs=1)
        msg_pool = tc.alloc_tile_pool(name="msg", bufs=3)
        gconst = tc.alloc_tile_pool(name="gconst", bufs=1)

        def load_gconst(name, shape, dtype=F32):
            t = gconst.tile(list(shape), dtype, name=f"g_{name}")
            nc.sync.dma_start(t[:], dram[name][:])
            return t

        We = {"mol": load_gconst("edge_lin_mol_W", [75, 64], BF16),
              "prot": load_gconst("edge_lin_prot_W", [75, 64], BF16)}
        sb_xTin = {"mol": load_gconst("mol_xT", [12, NC_MOL]),
                   "prot": load_gconst("prot_xT", [16, NC_PROT])}
        sb_seg = {s: load_gconst(f"{s}_segblk", [1, 2 * sides[s]["T_total"]],
                                 I32) for s in sides}
        sb_eaT, sb_ohmod, sb_ohdst = {}, {}, {}
        for s in sides:
            sd = sides[s]
            sb_eaT[s] = load_gconst(f"{s}_eaT", [75, meta[f"{s}_Wea"]], BF16)
            sb_ohmod[s] = load_gconst(f"{s}_ohmod", [128, sd["E_core"]], BF16)
            sb_ohdst[s] = load_gconst(f"{s}_ohdst",
                                      [128, sd["T_total"], 128], BF16)

        msgps = tc.alloc_tile_pool(name="msgps", bufs=2, space="PSUM")
        aggps = tc.alloc_tile_pool(name="aggps", bufs=2, space="PSUM")
        mlpps = tc.alloc_tile_pool(name="mlpps", bufs=2, space="PSUM")
        trps = tc.alloc_tile_pool(name="trps", bufs=2, space="PSUM")

        def xT_to_xnf_and_gather(s, l, xThi, xTlo=None):
            """Transpose xT shard to node-major, publish shard, AllGather.
            Last layer publishes hi|lo side by side in 128 feature cols."""
            sd = sides[s]
            nblk = sd["nblk"]
            fw = 64 if xTlo is None else 128
            xnf = xnf_pool.tile([128, nblk, fw], BF16, name=f"xnf_{s}_{fw}",
                                tag=f"xnf_{s}")
            for b in range(nblk):
                tp = trps.tile([128, 64], BF16, name="tr_ps")
                nc.tensor.transpose(tp[:], xThi[0:64, b * 128:(b + 1) * 128],
                                    ident_bf[0:64, 0:64])
                nc.vector.tensor_copy(xnf[:, b, 0:64], tp[:])
                if xTlo is not None:
                    tp2 = trps.tile([128, 64], BF16, name="tr_ps")
                    nc.tensor.transpose(tp2[:],
                                        xTlo[0:64, b * 128:(b + 1) * 128],
                                        ident_bf[0:64, 0:64])
                    nc.vector.tensor_copy(xnf[:, b, 64:128], tp2[:])
            nc.sync.dma_start(
                x_sh_d[s][l][:].rearrange("(t p) f -> p t f", p=128), xnf[:])
            nc.gpsimd.collective_compute(
                "AllGather", ALU.bypass, replica_groups=[list(range(R))],
                ins=[x_sh_d[s][l][:].opt()], outs=[x_full_d[s][l][:].opt()])
            return xnf

        def load_xfull(s, l):
            sd = sides[s]
            fw = 64 if l < 3 else 128
            xf = xfull_pool.tile([128, sd["nbf"], fw], BF16,
                                 name=f"xf_{s}_{fw}", tag=f"xf_{s}")
            nc.sync.dma_start(
                xf[:], x_full_d[s][l][:].rearrange("(t p) f -> p t f", p=128))
            return xf

        # initial node features x0 (no relu); x kept as f32 + bf16 hi/lo
        def make_triple(s, ps_ap, NCs, act):
            xTf = xT_pool.tile([65, NCs], F32, name=f"xTf_{s}", tag=f"xTf_{s}")
            nc.scalar.activation(xTf[0:64, :], ps_ap, act)
            xThi = xT_pool.tile([65, NCs], BF16, name=f"xTh_{s}",
                                tag=f"xTh_{s}")
            nc.scalar.activation(xThi[0:64, :], xTf[0:64, :], AF.Copy)
            xTlo = xT_pool.tile([65, NCs], BF16, name=f"xTl_{s}",
                                tag=f"xTl_{s}")
            nc.vector.tensor_sub(xTlo[0:64, :], xTf[0:64, :], xThi[0:64, :])
            nc.vector.memset(xThi[64:65, :], 1.0)
            nc.vector.memset(xTlo[64:65, :], 0.0)
            return xTf, xThi, xTlo

        xT_cur = {}
        xnf_cur = {}
        for s in sides:
            sd = sides[s]
            NCs = sd["NC"]
            ps = mlpps.tile([64, 512], F32, name="mlp_ps")
            nc.tensor.matmul(ps[:, 0:NCs], Wn[s][:], sb_xTin[s][:],
                             start=True, stop=True)
            xT_cur[s] = make_triple(s, ps[:, 0:NCs], NCs, AF.Copy)
            xnf_cur[s] = xT_to_xnf_and_gather(s, 0, xT_cur[s][1])

        # ---------------- GINE layers
        segregs = [nc.tensor.alloc_register(f"segreg{i}") for i in range(16)]

        def seg_vals16(ap, n, hi):
            regs = segregs[:n]
            nc.tensor.reg_load(regs, ap)
            return [nc.tensor.snap(r, donate=True, min_val=0, max_val=hi)
                    for r in regs]

        for l in range(3):
            for s in ("prot", "mol"):
                sd = sides[s]
                NCs, nblk, T_blk, nbf = sd["NC"], sd["nblk"], sd["T_blk"], sd["nbf"]
                xfull = load_xfull(s, l)
                xTf_prev = xT_cur[s][0]
                hTf = gmem.tile([65, NCs], F32, name=f"hTf_{s}",
                                tag=f"hTf_{s}")
                for b in range(nblk):
                    agg = aggps.tile([64, 128], F32, name="agg_ps")
                    for g0 in range(0, T_blk, 8):
                        ng = min(8, T_blk - g0)
                        t0 = b * T_blk + g0
                        vals = seg_vals16(
                            sb_seg[s][0:1, 2 * t0:2 * t0 + 2 * ng],
                            2 * ng, nbf - 1)
                        mps = msgps.tile([128, 8, 64], F32, name="msg_ps")
                        for j in range(ng):
                            t = t0 + j
                            e0 = t * 128
                            nc.tensor.matmul(
                                mps[0:64, j, :],
                                sb_ohmod[s][:, e0:e0 + 64],
                                xfull[:, ds(vals[2 * j], 1), :],
                                start=True, stop=False, tile_position=(0, 0),
                                skip_group_check=True)
                            nc.tensor.matmul(
                                mps[64:128, j, :],
                                sb_ohmod[s][:, e0 + 64:e0 + 128],
                                xfull[:, ds(vals[2 * j + 1], 1), :],
                                start=True, stop=False, tile_position=(0, 64),
                                skip_group_check=True)
                            rb, cb = 64 * (t % 2), 128 * (t // 2)
                            nc.tensor.matmul(
                                mps[:, j, :],
                                sb_eaT[s][rb:rb + 11, cb:cb + 128],
                                We[s][rb:rb + 11, :], start=False, stop=True,
                                skip_group_check=True)
                        msg = msg_pool.tile([128, 8, 64], BF16, name="msg_sb")
                        nc.scalar.activation(msg[:, 0:ng, :], mps[:, 0:ng, :],
                                             AF.Relu)
                        for j in range(ng):
                            t = b * T_blk + g0 + j
                            nc.tensor.matmul(
                                agg[:], msg[:, j, :], sb_ohdst[s][:, t, :],
                                start=(g0 + j == 0),
                                stop=(g0 + j == T_blk - 1),
                                skip_group_check=True)
                    nc.vector.tensor_add(hTf[0:64, b * 128:(b + 1) * 128],
                                         xTf_prev[0:64, b * 128:(b + 1) * 128],
                                         agg[:])
                hThi = gmem.tile([65, NCs], BF16, name=f"hTh_{s}",
                                 tag=f"hTh_{s}")
                nc.scalar.activation(hThi[0:64, :], hTf[0:64, :], AF.Copy)
                hTlo = gmem.tile([65, NCs], BF16, name=f"hTl_{s}",
                                 tag=f"hTl_{s}")
                nc.vector.tensor_sub(hTlo[0:64, :], hTf[0:64, :],
                                     hThi[0:64, :])
                nc.vector.memset(hThi[64:65, :], 1.0)
                nc.vector.memset(hTlo[64:65, :], 0.0)

                def mlp3(Wp, rhs_hi, rhs_lo, NCs):
                    ps_ = mlpps.tile([64, 512], F32, name="mlp_ps")
                    nc.tensor.matmul(ps_[:, 0:NCs], Wp[0][:], rhs_hi[:],
                                     start=True, stop=False,
                                     skip_group_check=True)
                    nc.tensor.matmul(ps_[:, 0:NCs], Wp[0][:], rhs_lo[:],
                                     start=False, stop=False,
                                     skip_group_check=True)
                    nc.tensor.matmul(ps_[:, 0:NCs], Wp[1][:], rhs_hi[:],
                                     start=False, stop=True,
                                     skip_group_check=True)
                    return ps_

                ps1 = mlp3(W1[s][l], hThi, hTlo, NCs)
                r1f = gmem.tile([65, NCs], F32, name=f"r1f_{s}",
                                tag=f"r1f_{s}")
                nc.scalar.activation(r1f[0:64, :], ps1[:, 0:NCs], AF.Relu)
                r1hi = gmem.tile([65, NCs], BF16, name=f"r1h_{s}",
                                 tag=f"r1h_{s}")
                nc.scalar.activation(r1hi[0:64, :], r1f[0:64, :], AF.Copy)
                r1lo = gmem.tile([65, NCs], BF16, name=f"r1l_{s}",
                                 tag=f"r1l_{s}")
                nc.vector.tensor_sub(r1lo[0:64, :], r1f[0:64, :],
                                     r1hi[0:64, :])
                nc.vector.memset(r1hi[64:65, :], 1.0)
                nc.vector.memset(r1lo[64:65, :], 0.0)
                ps2 = mlp3(W2[s][l], r1hi, r1lo, NCs)
                xT_cur[s] = make_triple(s, ps2[:, 0:NCs], NCs, AF.Relu)
                xnf_cur[s] = xT_to_xnf_and_gather(
                    s, l + 1, xT_cur[s][1],
                    xT_cur[s][2] if l == 2 else None)

        for p in (trps, mlpps, aggps, msgps):
            p.release()
        gconst.release()
        msg_pool.release()
        gmem.release()

        # ---------------- attention phase
        a_sb = tc.alloc_tile_pool(name="attn_sb", bufs=1)
        smallps = tc.alloc_tile_pool(name="smallps", bufs=2, space="PSUM")
        s12ps = tc.alloc_tile_pool(name="s12ps", bufs=2, space="PSUM")
        ops = tc.alloc_tile_pool(name="ops", bufs=1, space="PSUM")
        ex_pool = tc.alloc_tile_pool(name="expt", bufs=6)

        def sps():
            return smallps.tile([128, 512], F32, name="small_ps")

        def sbf():
            return smallps.tile([128, 128], BF16, name="small_bf")

        # final x of both sides: load node-major hi|lo, build transposed pair
        xT_full = {}
        for s in sides:
            sd = sides[s]
            Ns, nbf = sd["N"], sd["nbf"]
            xf = load_xfull(s, 3)
            xT_fh = a_sb.tile([65, Ns], BF16, name=f"xTfullh_{s}")
            xT_fl = a_sb.tile([65, Ns], BF16, name=f"xTfulll_{s}")
            for t in range(nbf):
                tp = sbf()
                nc.tensor.transpose(tp[0:64, 0:128], xf[:, t, 0:64],
                                    ident_bf[:])
                nc.vector.tensor_copy(xT_fh[0:64, t * 128:(t + 1) * 128],
                                      tp[0:64, 0:128])
                tp2 = sbf()
                nc.tensor.transpose(tp2[0:64, 0:128], xf[:, t, 64:128],
                                    ident_bf[:])
                nc.vector.tensor_copy(xT_fl[0:64, t * 128:(t + 1) * 128],
                                      tp2[0:64, 0:128])
            nc.vector.memset(xT_fh[64:65, :], 1.0)
            nc.vector.memset(xT_fl[64:65, :], 0.0)
            xT_full[s] = (xT_fh, xT_fl)

        H_sb = {}
        for dirn, (qs, ks) in (("mp", ("mol", "prot")), ("pm", ("prot", "mol"))):
            qd, kd = sides[qs], sides[ks]
            NCq, Nk = qd["NC"], kd["N"]
            n_qt = NCq // 128
            n_k128 = Nk // 128
            n_k512 = Nk // 512
            Wq, Wk, Wv = [], [], []
            for p, lst, wd in (("hi", Wq, 64), ("lo", Wq, 64),
                               ("hi", Wk, 68), ("lo", Wk, 68),
                               ("hi", Wv, 64), ("lo", Wv, 64)):
                nm = "q" if lst is Wq else ("k" if lst is Wk else "v")
                t = a_sb.tile([65, wd], BF16, name=f"W{nm}{p}_{dirn}")
                nc.sync.dma_start(t[:], dram[f"attn_{dirn}_W{nm}_{p}"][:])
                lst.append(t)

            def mm3w(out_ap, Wpair, csl, xpair, rsl, NCo):
                nc.tensor.matmul(out_ap, Wpair[0][:, csl], xpair[0][:, rsl],
                                 start=True, stop=False,
                                 skip_group_check=True)
                nc.tensor.matmul(out_ap, Wpair[0][:, csl], xpair[1][:, rsl],
                                 start=False, stop=False,
                                 skip_group_check=True)
                nc.tensor.matmul(out_ap, Wpair[1][:, csl], xpair[0][:, rsl],
                                 start=False, stop=True,
                                 skip_group_check=True)

            # K^T per head with ones row (via augmented Wk), hi/lo pair
            KTh = [a_sb.tile([81, Nk], BF16, name=f"KTh_{dirn}_{i}")
                   for i in range(2)]
            KTl = [a_sb.tile([81, Nk], BF16, name=f"KTl_{dirn}_{i}")
                   for i in range(2)]
            for h in range(HEADS):
                r0 = 64 * (h % 2)
                for cc in range(n_k512):
                    csl = slice(cc * 512, (cc + 1) * 512)
                    pk = sps()[r0:r0 + 17, :]
                    mm3w(pk[:], Wk, slice(17 * h, 17 * h + 17),
                         xT_full[ks], csl, 512)
                    nc.scalar.activation(KTh[h // 2][r0:r0 + 17, csl],
                                         pk[:], AF.Copy)
                    nc.vector.tensor_sub(KTl[h // 2][r0:r0 + 17, csl],
                                         pk[:], KTh[h // 2][r0:r0 + 17, csl])

            # Q^T per head (0.25 folded in Wq); hi has the -m row, plus lo
            QTo = [a_sb.tile([81, NCq], BF16, name=f"QTo_{dirn}_{i}")
                   for i in range(2)]
            for h in range(HEADS):
                r0 = 64 * (h % 2)
                pq = sps()[r0:r0 + 16, :]
                mm3w(pq[:, 0:NCq], Wq, slice(16 * h, 16 * h + 16),
                     (xT_cur[qs][1], xT_cur[qs][2]), slice(0, NCq), NCq)
                nc.scalar.activation(QTo[h // 2][r0:r0 + 16, :], pq[:, 0:NCq],
                                     AF.Copy)
            for h in range(HEADS):
                QTt, KTt, r0 = QTo[h // 2], KTh[h // 2], 64 * (h % 2)
                negm = a_sb.tile([1, NCq], BF16, name="negm", bufs=2,
                                 tag="negm")
                for qt in range(n_qt):
                    mx = a_sb.tile([128, n_k512], F32, name="mx", bufs=2,
                                   tag="mx")
                    for cch in range(n_k512):
                        spT = s12ps.tile([128, 512], F32, name="s_ps")
                        nc.tensor.matmul(
                            spT[:],
                            QTt[r0:r0 + 16, qt * 128:(qt + 1) * 128],
                            KTt[r0:r0 + 16, cch * 512:(cch + 1) * 512],
                            start=True, stop=True)
                        nc.vector.reduce_max(mx[:, cch:cch + 1], spT[:],
                                             axis=AX.X)
                    mqt = a_sb.tile([128, 1], F32, name="mqt", bufs=2,
                                    tag="mqt")
                    nc.vector.reduce_max(mqt[:], mx[:], axis=AX.X)
                    tpm = sps()[0:1, 0:128]
                    nc.tensor.transpose(tpm[:], mqt[:], ident_f32[:])
                    nc.scalar.activation(negm[0:1, qt * 128:(qt + 1) * 128],
                                         tpm[:], AF.Copy, scale=-1.0)
                nc.sync.dma_start(QTt[r0 + 16:r0 + 17, :], negm[:])

            # V' [128, n_k128, 4, 34]: cols 0-15 Vhi, 16 ones, 17-32 Vlo,
            # 33 zero; hi and lo halves feed two accumulating wV matmuls
            Vp = a_sb.tile([128, n_k128, HEADS, 34], BF16, name=f"Vp_{dirn}")
            nc.vector.memset(Vp[:, :, :, 16:17], 1.0)
            nc.vector.memset(Vp[:, :, :, 33:34], 0.0)
            for kt in range(n_k128):
                ksl = slice(kt * 128, (kt + 1) * 128)
                pv = sps()[:, 0:64]
                nc.tensor.matmul(pv[:], xT_full[ks][0][:, ksl], Wv[0][:],
                                 start=True, stop=False,
                                 skip_group_check=True)
                nc.tensor.matmul(pv[:], xT_full[ks][1][:, ksl], Wv[0][:],
                                 start=False, stop=False,
                                 skip_group_check=True)
                nc.tensor.matmul(pv[:], xT_full[ks][0][:, ksl], Wv[1][:],
                                 start=False, stop=True,
                                 skip_group_check=True)
                nc.scalar.activation(
                    Vp[:, kt, :, 0:16],
                    pv[:].rearrange("p (h d) -> p h d", h=HEADS), AF.Copy)
                nc.vector.tensor_sub(
                    Vp[:, kt, :, 17:33],
                    pv[:].rearrange("p (h d) -> p h d", h=HEADS),
                    Vp[:, kt, :, 0:16])

            # scores (hi.hi + hi.lo + lo.hi) -> exp -> wV, single pass
            o_ps = [ops.tile([81, NCq], F32, name=f"o_ps_{i}")
                    for i in range(2)]
            for kc in range(n_k128):
                ksl = slice(kc * 128, (kc + 1) * 128)
                for h in range(HEADS):
                    i2, r0 = h // 2, 64 * (h % 2)
                    sp = s12ps.tile([128, 512], F32, name="s_ps")[:, 0:NCq]
                    nc.tensor.matmul(sp[:], KTh[i2][r0:r0 + 17, ksl],
                                     QTo[i2][r0:r0 + 17, :],
                                     start=True, stop=False,
                                     skip_group_check=True)
                    nc.tensor.matmul(sp[:], KTl[i2][r0:r0 + 16, ksl],
                                     QTo[i2][r0:r0 + 16, :],
                                     start=False, stop=True,
                                     skip_group_check=True)
                    ex = ex_pool.tile([128, NCq], BF16, name="ex",
                                      tag=f"ex_{dirn}")
                    nc.scalar.activation(ex[:], sp[:], AF.Exp)
                    nc.tensor.matmul(o_ps[i2][r0:r0 + 17, :],
                                     Vp[:, kc, h, 0:17], ex[:],
                                     start=(kc == 0), stop=False,
                                     skip_group_check=True)
                    nc.tensor.matmul(o_ps[i2][r0:r0 + 17, :],
                                     Vp[:, kc, h, 17:34], ex[:],
                                     start=False, stop=(kc == n_k128 - 1),
                                     skip_group_check=True)

            # normalize + assemble H (node-major, f32) + residual hi+lo
            H = a_sb.tile([128, n_qt, 64], F32, name=f"H_{dirn}")
            for h in range(HEADS):
                ro = 64 * (h % 2)
                osb = a_sb.tile([81, NCq], F32, name="osb", bufs=2, tag="osb")
                nc.vector.tensor_copy(osb[ro:ro + 17, :],
                                      o_ps[h // 2][ro:ro + 17, :])
                for qt in range(n_qt):
                    tp = sps()[:, 0:17]
                    nc.tensor.transpose(tp[:],
                                        osb[ro:ro + 17,
                                            qt * 128:(qt + 1) * 128],
                                        ident_f32[ro:ro + 17, ro:ro + 17])
                    inv1 = a_sb.tile([128, 1], F32, name="inv1", bufs=2,
                                     tag="inv1")
                    nc.vector.reciprocal(inv1[:], tp[:, 16:17])
                    nc.vector.tensor_scalar_mul(
                        H[:, qt, 16 * h:16 * (h + 1)], tp[:, 0:16], inv1[:])
            nc.vector.tensor_add(H[:], H[:], xnf_cur[qs][:, :, 0:64])
            nc.vector.tensor_add(H[:], H[:], xnf_cur[qs][:, :, 64:128])
            H_sb[dirn] = H

        # ---------------- pooling + output MLP
        zt_part_d = dpool.tile([128, B], F32, name="zt_part")
        zt_full_d = dpool.tile([128, B], F32, addr_space="Shared",
                               name="zt_full")
        for dirn, qs in (("mp", "mol"), ("pm", "prot")):
            n_qt = sides[qs]["NC"] // 128
            psz = sps()[0:64, 0:B]
            for qt in range(n_qt):
                nc.tensor.matmul(psz[:], H_sb[dirn][:, qt, :],
                                 sb_pmat[qs][:, qt, :],
                                 start=(qt == 0), stop=(qt == n_qt - 1),
                                 skip_group_check=True)
            zpart = a_sb.tile([64, B], F32, name=f"zpart_{dirn}")
            nc.vector.tensor_copy(zpart[:], psz[:])
            row0 = 0 if dirn == "mp" else 64
            nc.sync.dma_start(zt_part_d[row0:row0 + 64, :], zpart[:])
        nc.gpsimd.collective_compute(
            "AllReduce", ALU.add, replica_groups=[list(range(R))],
            ins=[zt_part_d[:].opt()], outs=[zt_full_d[:].opt()])
        zT = a_sb.tile([128, B], F32, name="zT")
        nc.sync.dma_start(zT[:], zt_full_d[:])

        fc1W = a_sb.tile([128, 64], F32, name="fc1W")
        nc.sync.dma_start(fc1W[:], dram["fc1_W"][:])
        fc1b = a_sb.tile([64, 1], F32, name="fc1b")
        nc.sync.dma_start(fc1b[:], dram["fc1_b"][:, None])
        fc2W = a_sb.tile([64, 1], F32, name="fc2W")
        nc.sync.dma_start(fc2W[:], dram["fc2_W"][:])
        fc2b = a_sb.tile([1, 1], F32, name="fc2b")
        nc.sync.dma_start(fc2b[:], dram["fc2_b"][:, None])

        ps = sps()[0:64, 0:B]
        nc.tensor.matmul(ps[:], fc1W[:], zT[:], start=True, stop=True)
        h1 = a_sb.tile([65, B], F32, name="h1")
        nc.scalar.activation(h1[0:64, :], ps[:], AF.Relu, bias=fc1b[:])
        ps2 = sps()[0:1, 0:B]
        nc.tensor.matmul(ps2[:], fc2W[:], h1[0:64, :], start=True, stop=True)
        osb = a_sb.tile([1, B], F32, name="osb_out")
        nc.scalar.activation(osb[:], ps2[:], AF.Sigmoid, bias=fc2b[:])
        nc.sync.dma_start(out_d[:], osb[:])

        ex_pool.release()
        ops.release()
        s12ps.release()
        smallps.release()
        a_sb.release()
        xfull_pool.release()
        xnf_pool.release()
        xT_pool.release()
        dpool.release()
        const.release()

    nc.compile()
    return nc


# ----------------------------------------------------------------- entry

def kernel(**inputs):
    global last_results
    meta, percore = _prep_host(inputs)
    key = (meta["mol_T_blk"], meta["prot_T_blk"])
    if key not in _CACHE:
        _CACHE[key] = _build(meta)
    nc = _CACHE[key]
    from concourse.bass_utils import run_bass_kernel_spmd
    res = run_bass_kernel_spmd(nc, percore, list(range(R)))
    last_results = res
    return np.asarray(res.results[0]["out"], np.float32).reshape(B)


# revision 23
# speedup vs baseline: 1.3221x; 1.0022x over previous
"""CrossGraphAttentionModel on 8 Trainium2 NeuronCores (Bass/Tile, SPMD).

Sharding: nodes/edges of both graphs are sharded 8 ways by (dst-sorted) node
range; 64-dim weights replicated, all in bf16 on the PE. Per GINE layer the
x[src] gather is done ON the PE: edges are sorted by (dst window, src block)
with runs padded to 64, so every 64-edge half-tile reads one 128-node block
of the AllGathered x; a host-built src-mod-128 one-hot (lhsT) contracts
against that block, whose index is loaded per half-tile from a per-core table
into PE registers (dynamic rhs offset). The edge-linear term accumulates into
the same PSUM via a second matmul, ACT applies the relu, and a dst one-hot
matmul scatter-adds messages per 128-node window. Cross-graph attention is
single-pass: scores use contraction-17 matmuls (K^T tiles with a ones row
against per-head Q with a folded Cauchy-Schwarz row bound), exp runs on ACT
into bf16, and the wV product accumulates transposed [17, Nq] outputs with a
ones column producing softmax denominators for free. Graph pooling is a
one-hot matmul with 1/count weights, AllReduced, then the tiny output MLP.
"""

import numpy as np
import ml_dtypes

BF = ml_dtypes.bfloat16

R = 8
HID = 64
B = 32
HEADS = 4
HD = 16
N_MOL, N_PROT = 2048, 4096
E_MOL, E_PROT = 32768, 131072
NC_MOL, NC_PROT = N_MOL // R, N_PROT // R          # 256, 512
NBLK_MOL, NBLK_PROT = NC_MOL // 128, NC_PROT // 128  # 2, 4
NBF_MOL, NBF_PROT = N_MOL // 128, N_PROT // 128      # 16, 32

_CACHE = {}
last_results = None


# ----------------------------------------------------------------- host prep

def _prep_edges(edge_index, eattr, nblk):
    """Sort edges by dst window, then by src block inside each window, with
    every (window, srcblk) run padded to a multiple of 64 edges. Produces the
    per-core one-hot operands and the per-half-tile source-block table."""
    src, dst = np.asarray(edge_index[0]), np.asarray(edge_index[1])
    eattr = np.asarray(eattr, np.float32)
    D = eattr.shape[1]
    nwin = R * nblk

    order = np.argsort(dst, kind="stable")
    src_s, dst_s, ea_s = src[order], dst[order], eattr[order]
    win = dst_s // 128
    counts = np.bincount(win, minlength=nwin)
    starts = np.concatenate([[0], np.cumsum(counts)])

    # per-window edge lists grouped by srcblk, runs padded to 64
    win_srcmod = []
    win_dstloc = []
    win_ea = []
    win_blk64 = []   # srcblk per 64-edge slot
    padded_len = np.zeros(nwin, np.int64)
    for g in range(nwin):
        lo, hi = starts[g], starts[g + 1]
        s_g, d_g, e_g = src_s[lo:hi], dst_s[lo:hi], ea_s[lo:hi]
        o2 = np.argsort(s_g // 128, kind="stable")
        s_g, d_g, e_g = s_g[o2], d_g[o2], e_g[o2]
        blk = s_g // 128
        sm_l, dl_l, ea_l, b64_l = [], [], [], []
        for bk in np.unique(blk):
            m = blk == bk
            n = int(m.sum())
            npad = -n % 64
            sm = np.concatenate([s_g[m] % 128, np.full(npad, -1, np.int64)])
            dl = np.concatenate([d_g[m] - g * 128, np.full(npad, -1, np.int64)])
            ea = np.concatenate([e_g[m], np.zeros((npad, D), np.float32)])
            sm_l.append(sm); dl_l.append(dl); ea_l.append(ea)
            b64_l.extend([int(bk)] * ((n + npad) // 64))
        win_srcmod.append(np.concatenate(sm_l) if sm_l else np.zeros(0, np.int64))
        win_dstloc.append(np.concatenate(dl_l) if dl_l else np.zeros(0, np.int64))
        win_ea.append(np.concatenate(ea_l) if ea_l else np.zeros((0, D), np.float32))
        win_blk64.append(b64_l)
        padded_len[g] = len(win_srcmod[-1])

    T_blk = max(1, int(np.ceil(padded_len.max() / 128)))
    T_total = nblk * T_blk
    E_core = T_total * 128

    Wea = ((T_total + 1) // 2) * 128
    ohmod = np.zeros((R, 128, E_core), BF)
    ohdst = np.zeros((R, 128, T_total, 128), BF)
    eaT = np.zeros((R, D + 1, E_core), np.float32)
    segblk = np.zeros((R, 1, 2 * T_total), np.int32)
    for c in range(R):
        for b in range(nblk):
            g = c * nblk + b
            sm, dl, ea = win_srcmod[g], win_dstloc[g], win_ea[g]
            n = len(sm)
            off = b * T_blk * 128
            e_idx = off + np.arange(n)
            real = sm >= 0
            ohmod[c, sm[real], e_idx[real]] = 1.0
            t_idx = e_idx // 128
            p_idx = e_idx % 128
            ohdst[c, p_idx[real], t_idx[real], dl[real]] = 1.0
            eaT[c, :D, off:off + n] = ea.T
            eaT[c, D, off:off + n] = real.astype(np.float32)
            b64 = win_blk64[g]
            h0 = (b * T_blk) * 2
            segblk[c, 0, h0:h0 + len(b64)] = b64
    # pack eaT 2-up: tile t -> rows 64*(t%2)..+11, cols (t//2)*128..+128
    eaT_w = np.zeros((R, 75, Wea), np.float32)
    for t in range(T_total):
        rb, cb = 64 * (t % 2), 128 * (t // 2)
        eaT_w[:, rb:rb + D + 1, cb:cb + 128] = \
            eaT[:, :, t * 128:(t + 1) * 128]
    return dict(T_blk=T_blk, T_total=T_total, E_core=E_core, D=D, Wea=Wea,
                ohmod=ohmod, ohdst=ohdst,
                eaT_w=np.ascontiguousarray(eaT_w.astype(BF)), segblk=segblk)


def _prep_host(inp):
    """All integer/layout preprocessing. Returns (meta, per_core_inputs)."""
    mol = _prep_edges(inp["mol_edge_index"], inp["mol_eattr"], NBLK_MOL)
    prot = _prep_edges(inp["prot_edge_index"], inp["prot_eattr"], NBLK_PROT)

    # pool matrices with 1/count entries, wrapped [128, nblk, B]
    def pmat(batch, ncore, nblk):
        batch = np.asarray(batch)
        cnt = np.bincount(batch, minlength=B).astype(np.float32)
        inv = 1.0 / np.maximum(cnt, 1.0)
        m = np.zeros((R, ncore, B), np.float32)
        for c in range(R):
            sl = batch[c * ncore:(c + 1) * ncore]
            m[c, np.arange(ncore), sl] = inv[sl]
        return np.ascontiguousarray(
            m.reshape(R, nblk, 128, B).transpose(0, 2, 1, 3))

    mol_pmat = pmat(inp["mol_batch"], NC_MOL, NBLK_MOL)
    prot_pmat = pmat(inp["prot_batch"], NC_PROT, NBLK_PROT)

    # node features transposed per core with ones row (fp32)
    def xt(x, ncore):
        x = np.asarray(x, np.float32)
        d = x.shape[1]
        out = np.zeros((R, d + 1, ncore), np.float32)
        for c in range(R):
            out[c, :d] = x[c * ncore:(c + 1) * ncore].T
            out[c, d] = 1.0
        return out

    mol_xT = xt(inp["mol_x"], NC_MOL)        # [R, 12, 256]
    prot_xT = xt(inp["prot_x"], NC_PROT)     # [R, 16, 512]

    ident_bf = np.eye(128, dtype=BF)
    ident_f32 = np.eye(128, dtype=np.float32)

    def cat_wb(W, b):  # -> [din+1, dout] fp32
        W = np.asarray(W, np.float32)
        b = np.asarray(b, np.float32)
        return np.concatenate([W, b[None, :]], 0)

    percore = []
    for c in range(R):
        m = {
            "mol_xT": mol_xT[c], "prot_xT": prot_xT[c],
            "mol_eaT": mol["eaT_w"][c], "prot_eaT": prot["eaT_w"][c],
            "mol_ohmod": mol["ohmod"][c], "prot_ohmod": prot["ohmod"][c],
            "mol_ohdst": mol["ohdst"][c], "prot_ohdst": prot["ohdst"][c],
            "mol_segblk": mol["segblk"][c], "prot_segblk": prot["segblk"][c],
            "mol_pmat": mol_pmat[c], "prot_pmat": prot_pmat[c],
            "ident_bf": ident_bf, "ident_f32": ident_f32,
            "node_lin_mol_W": cat_wb(inp["node_lin_mol_W"], inp["node_lin_mol_b"]),
            "node_lin_prot_W": cat_wb(inp["node_lin_prot_W"], inp["node_lin_prot_b"]),
            "edge_lin_mol_W": np.ascontiguousarray(np.tile(
                np.pad(cat_wb(inp["edge_lin_mol_W"], inp["edge_lin_mol_b"]),
                       ((0, 53), (0, 0))), (2, 1))[:75]).astype(BF),
            "edge_lin_prot_W": np.ascontiguousarray(np.tile(
                np.pad(cat_wb(inp["edge_lin_prot_W"], inp["edge_lin_prot_b"]),
                       ((0, 53), (0, 0))), (2, 1))[:75]).astype(BF),
            "fc1_W": np.asarray(inp["fc1_W"], np.float32),
            "fc1_b": np.asarray(inp["fc1_b"], np.float32),
            "fc2_W": np.asarray(inp["fc2_W"], np.float32),
            "fc2_b": np.asarray(inp["fc2_b"], np.float32),
        }
        def hilo(w):
            hi = w.astype(BF)
            lo = (w - hi.astype(np.float32)).astype(BF)
            return hi, lo
        for s in ("mol", "prot"):
            for l in range(3):
                for nm, wf in (("W1", "b1"), ("W2", "b2")):
                    w = cat_wb(inp[f"{s}_conv_{nm}"][l], inp[f"{s}_conv_{wf}"][l])
                    hi, lo = hilo(w)
                    m[f"{s}_conv_{nm}_{l}_hi"] = hi
                    m[f"{s}_conv_{nm}_{l}_lo"] = lo
        for d in ("mp", "pm"):
            W = np.asarray(inp[f"attn_{d}_W"], np.float32)
            bb = np.asarray(inp[f"attn_{d}_b"], np.float32)
            wq = cat_wb(W[0], bb[0]) * 0.25
            wv = cat_wb(W[2], bb[2])
            wk = cat_wb(W[1], bb[1])
            wka = np.zeros((65, 68), np.float32)
            for h in range(HEADS):
                wka[:, 17 * h:17 * h + 16] = wk[:, 16 * h:16 * h + 16]
                wka[64, 17 * h + 16] = 1.0
            for nm, w in (("q", wq), ("k", wka), ("v", wv)):
                hi, lo = hilo(w)
                m[f"attn_{d}_W{nm}_hi"] = hi
                m[f"attn_{d}_W{nm}_lo"] = lo
        percore.append(m)

    meta = dict(mol_T_blk=mol["T_blk"], mol_T_total=mol["T_total"],
                mol_E_core=mol["E_core"], mol_Wea=mol["Wea"],
                prot_T_blk=prot["T_blk"], prot_T_total=prot["T_total"],
                prot_E_core=prot["E_core"], prot_Wea=prot["Wea"])
    return meta, percore


# ------------------------------------------------------------- device build

def _build(meta):
    import concourse.bacc as bacc
    import concourse.mybir as mybir
    import concourse.tile as tile
    from concourse.bass import ds

    F32 = mybir.dt.float32
    BF16 = mybir.dt.bfloat16
    I32 = mybir.dt.int32
    AF = mybir.ActivationFunctionType
    ALU = mybir.AluOpType
    AX = mybir.AxisListType

    nc = bacc.Bacc("TRN2", target_bir_lowering=False, debug=False,
                   num_devices=R)

    dram = {}

    def din(name, shape, dtype=F32):
        dram[name] = nc.dram_tensor(name, list(shape), dtype,
                                    kind="ExternalInput")
        return dram[name]

    mT, mE = meta["mol_T_total"], meta["mol_E_core"]
    pT, pE = meta["prot_T_total"], meta["prot_E_core"]

    din("mol_xT", [12, NC_MOL]); din("prot_xT", [16, NC_PROT])
    din("mol_eaT", [75, meta["mol_Wea"]], BF16)
    din("prot_eaT", [75, meta["prot_Wea"]], BF16)
    din("mol_ohmod", [128, mE], BF16); din("prot_ohmod", [128, pE], BF16)
    din("mol_ohdst", [128, mT, 128], BF16)
    din("prot_ohdst", [128, pT, 128], BF16)
    din("mol_segblk", [1, 2 * mT], I32); din("prot_segblk", [1, 2 * pT], I32)
    din("mol_pmat", [128, NBLK_MOL, B])
    din("prot_pmat", [128, NBLK_PROT, B])
    din("ident_bf", [128, 128], BF16); din("ident_f32", [128, 128])
    din("node_lin_mol_W", [12, 64]); din("node_lin_prot_W", [16, 64])
    din("edge_lin_mol_W", [75, 64], BF16)
    din("edge_lin_prot_W", [75, 64], BF16)
    for s in ("mol", "prot"):
        for l in range(3):
            for nm in ("W1", "W2"):
                din(f"{s}_conv_{nm}_{l}_hi", [65, 64], BF16)
                din(f"{s}_conv_{nm}_{l}_lo", [65, 64], BF16)
    for d in ("mp", "pm"):
        for p in ("hi", "lo"):
            din(f"attn_{d}_Wq_{p}", [65, 64], BF16)
            din(f"attn_{d}_Wk_{p}", [65, 68], BF16)
            din(f"attn_{d}_Wv_{p}", [65, 64], BF16)
    din("fc1_W", [128, 64]); din("fc1_b", [64])
    din("fc2_W", [64, 1]); din("fc2_b", [1])

    out_d = nc.dram_tensor("out", [1, B], F32, kind="ExternalOutput")

    sides = {
        "mol": dict(N=N_MOL, NC=NC_MOL, nblk=NBLK_MOL, nbf=NBF_MOL,
                    T_blk=meta["mol_T_blk"], T_total=mT, E_core=mE, din=12),
        "prot": dict(N=N_PROT, NC=NC_PROT, nblk=NBLK_PROT, nbf=NBF_PROT,
                     T_blk=meta["prot_T_blk"], T_total=pT, E_core=pE, din=16),
    }

    with tile.TileContext(nc) as tc:
        const = tc.alloc_tile_pool(name="const", bufs=1)

        def load_const(name, shape, dtype=F32):
            t = const.tile(list(shape), dtype, name=f"c_{name}")
            nc.sync.dma_start(t[:], dram[name][:])
            return t

        ident_bf = load_const("ident_bf", [128, 128], BF16)
        ident_f32 = load_const("ident_f32", [128, 128])

        Wn = {"mol": load_const("node_lin_mol_W", [12, 64]),
              "prot": load_const("node_lin_prot_W", [16, 64])}
        W1 = {s: [[load_const(f"{s}_conv_W1_{l}_{p}", [65, 64], BF16)
                   for p in ("hi", "lo")] for l in range(3)] for s in sides}
        W2 = {s: [[load_const(f"{s}_conv_W2_{l}_{p}", [65, 64], BF16)
                   for p in ("hi", "lo")] for l in range(3)] for s in sides}
        sb_pmat = {s: load_const(f"{s}_pmat", [128, sides[s]["nblk"], B])
                   for s in sides}

        # ---------------- DRAM internals (bf16 node-major x)
        dpool = tc.alloc_tile_pool(name="dram", bufs=1, space="DRAM")
        x_sh_d = {s: [dpool.tile([sides[s]["NC"], 64 if l < 3 else 128], BF16,
                                 name=f"xsh_{s}_{l}") for l in range(4)]
                  for s in sides}
        x_full_d = {s: [dpool.tile([sides[s]["N"], 64 if l < 3 else 128], BF16,
                                   addr_space="Shared", name=f"xfull_{s}_{l}")
                        for l in range(4)] for s in sides}

        # ---------------- SBUF pools
        xT_pool = tc.alloc_tile_pool(name="xT", bufs=2)
        xnf_pool = tc.alloc_tile_pool(name="xnf", bufs=2)
        xfull_pool = tc.alloc_tile_pool(name="xfull", bufs=1)
        gmem = tc.alloc_tile_pool(name="gmem", bufs=1)
        msg_pool = tc.alloc_tile_pool(name="msg", bufs=3)
        gconst = tc.alloc_tile_pool(name="gconst", bufs=1)

        def load_gconst(name, shape, dtype=F32):
            t = gconst.tile(list(shape), dtype, name=f"g_{name}")
            nc.sync.dma_start(t[:], dram[name][:])
            return t

        We = {"mol": load_gconst("edge_lin_mol_W", [75, 64], BF16),
              "prot": load_gconst("edge_lin_prot_W", [75, 64], BF16)}
        sb_xTin = {"mol": load_gconst("mol_xT", [12, NC_MOL]),
                   "prot": load_gconst("prot_xT", [16, NC_PROT])}
        sb_seg = {s: load_gconst(f"{s}_segblk", [1, 2 * sides[s]["T_total"]],
                                 I32) for s in sides}
        sb_eaT, sb_ohmod, sb_ohdst = {}, {}, {}
        for s in sides:
            sd = sides[s]
            Tb, nblk = sd["T_blk"], sd["nblk"]
            Wea = meta[f"{s}_Wea"]
            ea = gconst.tile([75, Wea], BF16, name=f"g_{s}_eaT")
            om = gconst.tile([128, sd["E_core"]], BF16, name=f"g_{s}_ohmod")
            od = gconst.tile([128, sd["T_total"], 128], BF16,
                             name=f"g_{s}_ohdst")
            for w in range(nblk):
                e0, e1 = w * Tb * 128, (w + 1) * Tb * 128
                nc.sync.dma_start(om[:, e0:e1], dram[f"{s}_ohmod"][:, e0:e1])
                nc.sync.dma_start(od[:, w * Tb:(w + 1) * Tb, :],
                                  dram[f"{s}_ohdst"][:, w * Tb:(w + 1) * Tb, :])
            h = (Wea // 2) // 128 * 128
            nc.sync.dma_start(ea[:, 0:h], dram[f"{s}_eaT"][:, 0:h])
            nc.sync.dma_start(ea[:, h:Wea], dram[f"{s}_eaT"][:, h:Wea])
            sb_eaT[s], sb_ohmod[s], sb_ohdst[s] = ea, om, od

        msgps = tc.alloc_tile_pool(name="msgps", bufs=2, space="PSUM")
        aggps = tc.alloc_tile_pool(name="aggps", bufs=2, space="PSUM")
        mlpps = tc.alloc_tile_pool(name="mlpps", bufs=2, space="PSUM")
        trps = tc.alloc_tile_pool(name="trps", bufs=2, space="PSUM")

        def xT_to_xnf_and_gather(s, l, xThi, xTlo=None):
            """Transpose xT shard to node-major, publish shard, AllGather.
            Last layer publishes hi|lo side by side in 128 feature cols."""
            sd = sides[s]
            nblk = sd["nblk"]
            fw = 64 if xTlo is None else 128
            xnf = xnf_pool.tile([128, nblk, fw], BF16, name=f"xnf_{s}_{fw}",
                                tag=f"xnf_{s}")
            for b in range(nblk):
                tp = trps.tile([128, 64], BF16, name="tr_ps")
                nc.tensor.transpose(tp[:], xThi[0:64, b * 128:(b + 1) * 128],
                                    ident_bf[0:64, 0:64])
                nc.vector.tensor_copy(xnf[:, b, 0:64], tp[:])
                if xTlo is not None:
                    tp2 = trps.tile([128, 64], BF16, name="tr_ps")
                    nc.tensor.transpose(tp2[:],
                                        xTlo[0:64, b * 128:(b + 1) * 128],
                                        ident_bf[0:64, 0:64])
                    nc.vector.tensor_copy(xnf[:, b, 64:128], tp2[:])
            nc.sync.dma_start(
                x_sh_d[s][l][:].rearrange("(t p) f -> p t f", p=128), xnf[:])
            nc.gpsimd.collective_compute(
                "AllGather", ALU.bypass, replica_groups=[list(range(R))],
                ins=[x_sh_d[s][l][:].opt()], outs=[x_full_d[s][l][:].opt()])
            return xnf

        def load_xfull(s, l):
            sd = sides[s]
            fw = 64 if l < 3 else 128
            xf = xfull_pool.tile([128, sd["nbf"], fw], BF16,
                                 name=f"xf_{s}_{fw}", tag=f"xf_{s}")
            nc.sync.dma_start(
                xf[:], x_full_d[s][l][:].rearrange("(t p) f -> p t f", p=128))
            return xf

        # initial node features x0 (no relu); x kept as f32 + bf16 hi/lo
        def make_triple(s, ps_ap, NCs, act):
            xTf = xT_pool.tile([65, NCs], F32, name=f"xTf_{s}", tag=f"xTf_{s}")
            nc.scalar.activation(xTf[0:64, :], ps_ap, act)
            xThi = xT_pool.tile([65, NCs], BF16, name=f"xTh_{s}",
                                tag=f"xTh_{s}")
            nc.scalar.activation(xThi[0:64, :], xTf[0:64, :], AF.Copy)
            xTlo = xT_pool.tile([65, NCs], BF16, name=f"xTl_{s}",
                                tag=f"xTl_{s}")
            nc.vector.tensor_sub(xTlo[0:64, :], xTf[0:64, :], xThi[0:64, :])
            nc.vector.memset(xThi[64:65, :], 1.0)
            nc.vector.memset(xTlo[64:65, :], 0.0)
            return xTf, xThi, xTlo

        xT_cur = {}
        xnf_cur = {}
        for s in sides:
            sd = sides[s]
            NCs = sd["NC"]
            ps = mlpps.tile([64, 512], F32, name="mlp_ps")
            nc.tensor.matmul(ps[:, 0:NCs], Wn[s][:], sb_xTin[s][:],
                             start=True, stop=True)
            xT_cur[s] = make_triple(s, ps[:, 0:NCs], NCs, AF.Copy)
            xnf_cur[s] = xT_to_xnf_and_gather(s, 0, xT_cur[s][1])

        # ---------------- GINE layers
        segregs = [nc.tensor.alloc_register(f"segreg{i}") for i in range(32)]
        seg_grp = [0]

        def seg_vals16(ap, n, hi):
            base = 16 * (seg_grp[0] % 2)
            seg_grp[0] += 1
            regs = segregs[base:base + n]
            nc.tensor.reg_load(regs, ap)
            return [nc.tensor.snap(r, donate=True, min_val=0, max_val=hi)
                    for r in regs]

        for l in range(3):
            for s in ("prot", "mol"):
                sd = sides[s]
                NCs, nblk, T_blk, nbf = sd["NC"], sd["nblk"], sd["T_blk"], sd["nbf"]
                xfull = load_xfull(s, l)
                xTf_prev = xT_cur[s][0]
                hTf = gmem.tile([65, NCs], F32, name=f"hTf_{s}",
                                tag=f"hTf_{s}")
                for b in range(nblk):
                    agg = aggps.tile([64, 128], F32, name="agg_ps")
                    for g0 in range(0, T_blk, 8):
                        ng = min(8, T_blk - g0)
                        t0 = b * T_blk + g0
                        vals = seg_vals16(
                            sb_seg[s][0:1, 2 * t0:2 * t0 + 2 * ng],
                            2 * ng, nbf - 1)
                        mps = msgps.tile([128, 8, 64], F32, name="msg_ps")
                        for j in range(ng):
                            t = t0 + j
                            e0 = t * 128
                            nc.tensor.matmul(
                                mps[0:64, j, :],
                                sb_ohmod[s][:, e0:e0 + 64],
                                xfull[:, ds(vals[2 * j], 1), :],
                                start=True, stop=False, tile_position=(0, 0),
                                skip_group_check=True)
                            nc.tensor.matmul(
                                mps[64:128, j, :],
                                sb_ohmod[s][:, e0 + 64:e0 + 128],
                                xfull[:, ds(vals[2 * j + 1], 1), :],
                                start=True, stop=False, tile_position=(0, 64),
                                skip_group_check=True)
                            rb, cb = 64 * (t % 2), 128 * (t // 2)
                            nc.tensor.matmul(
                                mps[:, j, :],
                                sb_eaT[s][rb:rb + 11, cb:cb + 128],
                                We[s][rb:rb + 11, :], start=False, stop=True,
                                skip_group_check=True)
                        msg = msg_pool.tile([128, 8, 64], BF16, name="msg_sb")
                        nc.scalar.activation(msg[:, 0:ng, :], mps[:, 0:ng, :],
                                             AF.Relu)
                        for j in range(ng):
                            t = b * T_blk + g0 + j
                            nc.tensor.matmul(
                                agg[:], msg[:, j, :], sb_ohdst[s][:, t, :],
                                start=(g0 + j == 0),
                                stop=(g0 + j == T_blk - 1),
                                skip_group_check=True)
                    nc.vector.tensor_add(hTf[0:64, b * 128:(b + 1) * 128],
                                         xTf_prev[0:64, b * 128:(b + 1) * 128],
                                         agg[:])
                hThi = gmem.tile([65, NCs], BF16, name=f"hTh_{s}",
                                 tag=f"hTh_{s}")
                nc.scalar.activation(hThi[0:64, :], hTf[0:64, :], AF.Copy)
                hTlo = gmem.tile([65, NCs], BF16, name=f"hTl_{s}",
                                 tag=f"hTl_{s}")
                nc.vector.tensor_sub(hTlo[0:64, :], hTf[0:64, :],
                                     hThi[0:64, :])
                nc.vector.memset(hThi[64:65, :], 1.0)
                nc.vector.memset(hTlo[64:65, :], 0.0)

                def mlp3(Wp, rhs_hi, rhs_lo, NCs):
                    ps_ = mlpps.tile([64, 512], F32, name="mlp_ps")
                    nc.tensor.matmul(ps_[:, 0:NCs], Wp[0][:], rhs_hi[:],
                                     start=True, stop=False,
                                     skip_group_check=True)
                    nc.tensor.matmul(ps_[:, 0:NCs], Wp[0][:], rhs_lo[:],
                                     start=False, stop=False,
                                     skip_group_check=True)
                    nc.tensor.matmul(ps_[:, 0:NCs], Wp[1][:], rhs_hi[:],
                                     start=False, stop=True,
                                     skip_group_check=True)
                    return ps_

                ps1 = mlp3(W1[s][l], hThi, hTlo, NCs)
                r1f = gmem.tile([65, NCs], F32, name=f"r1f_{s}",
                                tag=f"r1f_{s}")
                nc.scalar.activation(r1f[0:64, :], ps1[:, 0:NCs], AF.Relu)
                r1hi = gmem.tile([65, NCs], BF16, name=f"r1h_{s}",
                                 tag=f"r1h_{s}")
                nc.scalar.activation(r1hi[0:64, :], r1f[0:64, :], AF.Copy)
                r1lo = gmem.tile([65, NCs], BF16, name=f"r1l_{s}",
                                 tag=f"r1l_{s}")
                nc.vector.tensor_sub(r1lo[0:64, :], r1f[0:64, :],
                                     r1hi[0:64, :])
                nc.vector.memset(r1hi[64:65, :], 1.0)
                nc.vector.memset(r1lo[64:65, :], 0.0)
                ps2 = mlp3(W2[s][l], r1hi, r1lo, NCs)
                xT_cur[s] = make_triple(s, ps2[:, 0:NCs], NCs, AF.Relu)
                xnf_cur[s] = xT_to_xnf_and_gather(
                    s, l + 1, xT_cur[s][1],
                    xT_cur[s][2] if l == 2 else None)

        for p in (trps, mlpps, aggps, msgps):
            p.release()
        gconst.release()
        msg_pool.release()
        gmem.release()

        # ---------------- attention phase
        a_sb = tc.alloc_tile_pool(name="attn_sb", bufs=1)
        smallps = tc.alloc_tile_pool(name="smallps", bufs=2, space="PSUM")
        s12ps = tc.alloc_tile_pool(name="s12ps", bufs=2, space="PSUM")
        ops = tc.alloc_tile_pool(name="ops", bufs=1, space="PSUM")
        ex_pool = tc.alloc_tile_pool(name="expt", bufs=6)

        def sps():
            return smallps.tile([128, 512], F32, name="small_ps")

        def sbf():
            return smallps.tile([128, 128], BF16, name="small_bf")

        # final x of both sides: load node-major hi|lo, build transposed pair
        xT_full = {}
        for s in sides:
            sd = sides[s]
            Ns, nbf = sd["N"], sd["nbf"]
            xf = load_xfull(s, 3)
            xT_fh = a_sb.tile([65, Ns], BF16, name=f"xTfullh_{s}")
            xT_fl = a_sb.tile([65, Ns], BF16, name=f"xTfulll_{s}")
            for t in range(nbf):
                tp = sbf()
                nc.tensor.transpose(tp[0:64, 0:128], xf[:, t, 0:64],
                                    ident_bf[:])
                nc.vector.tensor_copy(xT_fh[0:64, t * 128:(t + 1) * 128],
                                      tp[0:64, 0:128])
                tp2 = sbf()
                nc.tensor.transpose(tp2[0:64, 0:128], xf[:, t, 64:128],
                                    ident_bf[:])
                nc.vector.tensor_copy(xT_fl[0:64, t * 128:(t + 1) * 128],
                                      tp2[0:64, 0:128])
            nc.vector.memset(xT_fh[64:65, :], 1.0)
            nc.vector.memset(xT_fl[64:65, :], 0.0)
            xT_full[s] = (xT_fh, xT_fl)

        H_sb = {}
        for dirn, (qs, ks) in (("mp", ("mol", "prot")), ("pm", ("prot", "mol"))):
            qd, kd = sides[qs], sides[ks]
            NCq, Nk = qd["NC"], kd["N"]
            n_qt = NCq // 128
            n_k128 = Nk // 128
            n_k512 = Nk // 512
            Wq, Wk, Wv = [], [], []
            for p, lst, wd in (("hi", Wq, 64), ("lo", Wq, 64),
                               ("hi", Wk, 68), ("lo", Wk, 68),
                               ("hi", Wv, 64), ("lo", Wv, 64)):
                nm = "q" if lst is Wq else ("k" if lst is Wk else "v")
                t = a_sb.tile([65, wd], BF16, name=f"W{nm}{p}_{dirn}")
                nc.sync.dma_start(t[:], dram[f"attn_{dirn}_W{nm}_{p}"][:])
                lst.append(t)

            def mm3w(out_ap, Wpair, csl, xpair, rsl, NCo):
                nc.tensor.matmul(out_ap, Wpair[0][:, csl], xpair[0][:, rsl],
                                 start=True, stop=False,
                                 skip_group_check=True)
                nc.tensor.matmul(out_ap, Wpair[0][:, csl], xpair[1][:, rsl],
                                 start=False, stop=False,
                                 skip_group_check=True)
                nc.tensor.matmul(out_ap, Wpair[1][:, csl], xpair[0][:, rsl],
                                 start=False, stop=True,
                                 skip_group_check=True)

            # K^T per head with ones row (via augmented Wk), hi/lo pair
            KTh = [a_sb.tile([81, Nk], BF16, name=f"KTh_{dirn}_{i}")
                   for i in range(2)]
            KTl = [a_sb.tile([81, Nk], BF16, name=f"KTl_{dirn}_{i}")
                   for i in range(2)]
            for h in range(HEADS):
                r0 = 64 * (h % 2)
                for cc in range(n_k512):
                    csl = slice(cc * 512, (cc + 1) * 512)
                    pk = sps()[r0:r0 + 17, :]
                    mm3w(pk[:], Wk, slice(17 * h, 17 * h + 17),
                         xT_full[ks], csl, 512)
                    nc.scalar.activation(KTh[h // 2][r0:r0 + 17, csl],
                                         pk[:], AF.Copy)
                    nc.vector.tensor_sub(KTl[h // 2][r0:r0 + 17, csl],
                                         pk[:], KTh[h // 2][r0:r0 + 17, csl])

            # Q^T per head (0.25 folded in Wq); hi has the -m row, plus lo
            QTo = [a_sb.tile([81, NCq], BF16, name=f"QTo_{dirn}_{i}")
                   for i in range(2)]
            for h in range(HEADS):
                r0 = 64 * (h % 2)
                pq = sps()[r0:r0 + 16, :]
                mm3w(pq[:, 0:NCq], Wq, slice(16 * h, 16 * h + 16),
                     (xT_cur[qs][1], xT_cur[qs][2]), slice(0, NCq), NCq)
                nc.scalar.activation(QTo[h // 2][r0:r0 + 16, :], pq[:, 0:NCq],
                                     AF.Copy)
            for h in range(HEADS):
                QTt, KTt, r0 = QTo[h // 2], KTh[h // 2], 64 * (h % 2)
                negm = a_sb.tile([1, NCq], BF16, name="negm", bufs=2,
                                 tag="negm")
                for qt in range(n_qt):
                    mx = a_sb.tile([128, n_k512], F32, name="mx", bufs=2,
                                   tag="mx")
                    for cch in range(n_k512):
                        spT = s12ps.tile([128, 512], F32, name="s_ps")
                        nc.tensor.matmul(
                            spT[:],
                            QTt[r0:r0 + 16, qt * 128:(qt + 1) * 128],
                            KTt[r0:r0 + 16, cch * 512:(cch + 1) * 512],
                            start=True, stop=True)
                        nc.vector.reduce_max(mx[:, cch:cch + 1], spT[:],
                                             axis=AX.X)
                    mqt = a_sb.tile([128, 1], F32, name="mqt", bufs=2,
                                    tag="mqt")
                    nc.vector.reduce_max(mqt[:], mx[:], axis=AX.X)
                    tpm = sps()[0:1, 0:128]
                    nc.tensor.transpose(tpm[:], mqt[:], ident_f32[:])
                    nc.scalar.activation(negm[0:1, qt * 128:(qt + 1) * 128],
                                         tpm[:], AF.Copy, scale=-1.0)
                nc.sync.dma_start(QTt[r0 + 16:r0 + 17, :], negm[:])

            # V' [128, n_k128, 4, 34]: cols 0-15 Vhi, 16 ones, 17-32 Vlo,
            # 33 zero; hi and lo halves feed two accumulating wV matmuls
            Vp = a_sb.tile([128, n_k128, HEADS, 34], BF16, name=f"Vp_{dirn}")
            nc.vector.memset(Vp[:, :, :, 16:17], 1.0)
            nc.vector.memset(Vp[:, :, :, 33:34], 0.0)
            for kt in range(n_k128):
                ksl = slice(kt * 128, (kt + 1) * 128)
                pv = sps()[:, 0:64]
                nc.tensor.matmul(pv[:], xT_full[ks][0][:, ksl], Wv[0][:],
                                 start=True, stop=False,
                                 skip_group_check=True)
                nc.tensor.matmul(pv[:], xT_full[ks][1][:, ksl], Wv[0][:],
                                 start=False, stop=False,
                                 skip_group_check=True)
                nc.tensor.matmul(pv[:], xT_full[ks][0][:, ksl], Wv[1][:],
                                 start=False, stop=True,
                                 skip_group_check=True)
                nc.scalar.activation(
                    Vp[:, kt, :, 0:16],
                    pv[:].rearrange("p (h d) -> p h d", h=HEADS), AF.Copy)
                nc.vector.tensor_sub(
                    Vp[:, kt, :, 17:33],
                    pv[:].rearrange("p (h d) -> p h d", h=HEADS),
                    Vp[:, kt, :, 0:16])

            # scores (hi.hi + hi.lo + lo.hi) -> exp -> wV, single pass
            o_ps = [ops.tile([81, NCq], F32, name=f"o_ps_{i}")
                    for i in range(2)]
            for kc in range(n_k128):
                ksl = slice(kc * 128, (kc + 1) * 128)
                for h in range(HEADS):
                    i2, r0 = h // 2, 64 * (h % 2)
                    sp = s12ps.tile([128, 512], F32, name="s_ps")[:, 0:NCq]
                    nc.tensor.matmul(sp[:], KTh[i2][r0:r0 + 17, ksl],
                                     QTo[i2][r0:r0 + 17, :],
                                     start=True, stop=False,
                                     skip_group_check=True)
                    nc.tensor.matmul(sp[:], KTl[i2][r0:r0 + 16, ksl],
                                     QTo[i2][r0:r0 + 16, :],
                                     start=False, stop=True,
                                     skip_group_check=True)
                    ex = ex_pool.tile([128, NCq], BF16, name="ex",
                                      tag=f"ex_{dirn}")
                    nc.scalar.activation(ex[:], sp[:], AF.Exp)
                    nc.tensor.matmul(o_ps[i2][r0:r0 + 17, :],
                                     Vp[:, kc, h, 0:17], ex[:],
                                     start=(kc == 0), stop=False,
                                     skip_group_check=True)
                    nc.tensor.matmul(o_ps[i2][r0:r0 + 17, :],
                                     Vp[:, kc, h, 17:34], ex[:],
                                     start=False, stop=(kc == n_k128 - 1),
                                     skip_group_check=True)

            # normalize + assemble H (node-major, f32) + residual hi+lo
            H = a_sb.tile([128, n_qt, 64], F32, name=f"H_{dirn}")
            for h in range(HEADS):
                ro = 64 * (h % 2)
                osb = a_sb.tile([81, NCq], F32, name="osb", bufs=2, tag="osb")
                nc.vector.tensor_copy(osb[ro:ro + 17, :],
                                      o_ps[h // 2][ro:ro + 17, :])
                for qt in range(n_qt):
                    tp = sps()[:, 0:17]
                    nc.tensor.transpose(tp[:],
                                        osb[ro:ro + 17,
                                            qt * 128:(qt + 1) * 128],
                                        ident_f32[ro:ro + 17, ro:ro + 17])
                    inv1 = a_sb.tile([128, 1], F32, name="inv1", bufs=2,
                                     tag="inv1")
                    nc.vector.reciprocal(inv1[:], tp[:, 16:17])
                    nc.vector.tensor_scalar_mul(
                        H[:, qt, 16 * h:16 * (h + 1)], tp[:, 0:16], inv1[:])
            nc.vector.tensor_add(H[:], H[:], xnf_cur[qs][:, :, 0:64])
            nc.vector.tensor_add(H[:], H[:], xnf_cur[qs][:, :, 64:128])
            H_sb[dirn] = H

        # ---------------- pooling + output MLP
        zt_part_d = dpool.tile([128, B], F32, name="zt_part")
        zt_full_d = dpool.tile([128, B], F32, addr_space="Shared",
                               name="zt_full")
        for dirn, qs in (("mp", "mol"), ("pm", "prot")):
            n_qt = sides[qs]["NC"] // 128
            psz = sps()[0:64, 0:B]
            for qt in range(n_qt):
                nc.tensor.matmul(psz[:], H_sb[dirn][:, qt, :],
                                 sb_pmat[qs][:, qt, :],
                                 start=(qt == 0), stop=(qt == n_qt - 1),
                                 skip_group_check=True)
            zpart = a_sb.tile([64, B], F32, name=f"zpart_{dirn}")
            nc.vector.tensor_copy(zpart[:], psz[:])
            row0 = 0 if dirn == "mp" else 64
            nc.sync.dma_start(zt_part_d[row0:row0 + 64, :], zpart[:])
        nc.gpsimd.collective_compute(
            "AllReduce", ALU.add, replica_groups=[list(range(R))],
            ins=[zt_part_d[:].opt()], outs=[zt_full_d[:].opt()])
        zT = a_sb.tile([128, B], F32, name="zT")
        nc.sync.dma_start(zT[:], zt_full_d[:])

        fc1W = a_sb.tile([128, 64], F32, name="fc1W")
        nc.sync.dma_start(fc1W[:], dram["fc1_W"][:])
        fc1b = a_sb.tile([64, 1], F32, name="fc1b")
        nc.sync.dma_start(fc1b[:], dram["fc1_b"][:, None])
        fc2W = a_sb.tile([64, 1], F32, name="fc2W")
        nc.sync.dma_start(fc2W[:], dram["fc2_W"][:])
        fc2b = a_sb.tile([1, 1], F32, name="fc2b")
        nc.sync.dma_start(fc2b[:], dram["fc2_b"][:, None])

        ps = sps()[0:64, 0:B]
        nc.tensor.matmul(ps[:], fc1W[:], zT[:], start=True, stop=True)
        h1 = a_sb.tile([65, B], F32, name="h1")
        nc.scalar.activation(h1[0:64, :], ps[:], AF.Relu, bias=fc1b[:])
        ps2 = sps()[0:1, 0:B]
        nc.tensor.matmul(ps2[:], fc2W[:], h1[0:64, :], start=True, stop=True)
        osb = a_sb.tile([1, B], F32, name="osb_out")
        nc.scalar.activation(osb[:], ps2[:], AF.Sigmoid, bias=fc2b[:])
        nc.sync.dma_start(out_d[:], osb[:])

        ex_pool.release()
        ops.release()
        s12ps.release()
        smallps.release()
        a_sb.release()
        xfull_pool.release()
        xnf_pool.release()
        xT_pool.release()
        dpool.release()
        const.release()

    nc.compile()
    return nc


# ----------------------------------------------------------------- entry

def kernel(**inputs):
    global last_results
    meta, percore = _prep_host(inputs)
    key = (meta["mol_T_blk"], meta["prot_T_blk"])
    if key not in _CACHE:
        _CACHE[key] = _build(meta)
    nc = _CACHE[key]
    from concourse.bass_utils import run_bass_kernel_spmd
    res = run_bass_kernel_spmd(nc, percore, list(range(R)))
    last_results = res
    return np.asarray(res.results[0]["out"], np.float32).reshape(B)
